# revision 25
# baseline (speedup 1.0000x reference)
# Trainium2 Bass kernel for nn_Network_515396076038 (nms_detection / OICR-style loss).
#
# Strategy (8 NeuronCores, data-parallel over the N=4096 proposals):
#   - Each core owns NS = N/8 = 512 rois and streams its shard of fc7_roi /
#     fc7_frame / fc7_context from HBM in bf16 (12 MB per core -> memory
#     roofline ~34us), computing the GEMM heads in class-major layout [C, NS]
#     (scores^T = W^T @ X^T accumulated over F).  det = W@frm - W@ctx is
#     computed on the PE by accumulating ctx with -W (host-negated weights),
#     so the vector engine does no work during the stream.
#   - Per-class argmax over rois is computed in LOG domain:
#     log p = cls + det + ln(isw) - ln(sum_c exp(cls)) -- the global det
#     softmax normalizer is a per-class constant and cannot change the
#     argmax, and the log form avoids expensive vector reciprocals.
#   - One AllGather ships each core's per-class maxima + candidate boxes
#     (sel-mask matmul gather) + softmax partial sums; refine-head
#     log-softmax prep fills the collective latency.  A final tiny
#     AllReduce(add) sums 16 loss partials (4 roi-blocks x 2 supervisions x
#     {num, den}); the per-block sums ride the collective for free.
#   - IoU / fg-bg assignment / one-hot log-prob gather run roi-major,
#     batched across all 4 roi-blocks and both supervisions in single
#     vector ops on [128, 160] tiles via stride-0 broadcast views.
import sys

for _p in ("/opt/trn_rl_repo",):
    if _p not in sys.path:
        sys.path.append(_p)

import numpy as np
import ml_dtypes

import concourse.bass as bass
import concourse.bass_isa as bass_isa
import concourse.mybir as mybir
import concourse.tile as tile
from concourse import bacc
from concourse.bass_utils import run_bass_kernel_spmd
from concourse.masks import make_identity

dt = mybir.dt
Alu = mybir.AluOpType
Act = mybir.ActivationFunctionType
AX = mybir.AxisListType

C = 20      # foreground classes
CR = C + 1  # refine head classes (background + C)
CA = C + 2 * CR  # stacked roi-head outputs: cls | r1 | r2 = 62
W2 = 2 * C


def _emit(nc, tc, aps, NS, F, n_cores):
    NB = NS // 128
    KT = F // 128
    group = [list(range(n_cores))]
    GW = 241  # AllGather row: vm1[20] vm2[21] boxes[160] z[20] s1[20]

    (roi, frm, ctxm, w_all, wdc, b_all, b_det, boxes, isw, iswc, lab,
     loss) = aps

    const = tc.alloc_tile_pool(name="const", bufs=1)
    st = tc.alloc_tile_pool(name="st", bufs=1)
    stp = tc.alloc_tile_pool(name="stp", bufs=2)
    natp = tc.alloc_tile_pool(name="natp", bufs=2)
    pst = tc.alloc_tile_pool(name="pst", bufs=2, space="PSUM")
    pss1 = tc.alloc_tile_pool(name="pss1", bufs=2, space="PSUM")
    pss2 = tc.alloc_tile_pool(name="pss2", bufs=1, space="PSUM")
    dp = tc.alloc_tile_pool(name="dp", bufs=1, space="DRAM")
    # psc is created last: it is the first pool released (LIFO pool stack)
    psc = tc.alloc_tile_pool(name="psc", bufs=1, space="PSUM")

    # ---- dummy collective: absorbs CC-engine warmup under the DMA stream
    dz = const.tile([1, 1], dt.float32)
    nc.vector.memset(dz, 0.0)
    cc0_in = dp.tile([4], dt.float32)
    cc0_out = dp.tile([4 * n_cores], dt.float32)
    nc.sync.dma_start(cc0_in[0:1], dz[:, 0])
    nc.sync.dma_start(cc0_in[1:2], dz[:, 0])
    nc.sync.dma_start(cc0_in[2:3], dz[:, 0])
    nc.sync.dma_start(cc0_in[3:4], dz[:, 0])
    nc.gpsimd.collective_compute(
        "AllGather", Alu.bypass, replica_groups=group,
        ins=[cc0_in.opt()], outs=[cc0_out.opt()],
    )

    # ---- big stream DMAs first: weights + first super-tile ----
    KS = min(8, KT)
    SK = KT // KS
    w_all_sb = const.tile([128, KT, CA], dt.bfloat16)
    nc.sync.dma_start(w_all_sb, w_all)
    wdc_sb = const.tile([128, KT, 2, C], dt.bfloat16)
    nc.sync.dma_start(wdc_sb, wdc)
    nat0 = []
    for nm, srcp in (("roi", roi), ("frm", frm), ("ctx", ctxm)):
        t = natp.tile([128, KS, NS], dt.bfloat16, tag=nm)
        nc.sync.dma_start(t, srcp[:, bass.ts(0, KS), :])
        nat0.append(t)

    # ---------------- constants / setup (overlaps the GEMM stream) --------
    ident = const.tile([128, 128], dt.float32)
    make_identity(nc, ident)
    ones_col = const.tile([128, 1], dt.float32)
    nc.vector.memset(ones_col, 1.0)
    ones_row = const.tile([1, 128], dt.float32)
    nc.vector.memset(ones_row, 1.0)
    iota_i = const.tile([128, CR], dt.int32)
    nc.gpsimd.iota(iota_i, pattern=[[1, CR]], base=0, channel_multiplier=0)
    iota_f = const.tile([128, CR], dt.float32)
    nc.vector.tensor_copy(iota_f, iota_i)
    iota_m1k = const.tile([128, C], dt.float32)
    nc.vector.tensor_scalar_add(iota_m1k, iota_f[:, :C], -1000.0)
    iota8 = const.tile([128, 2 * NB * CR], dt.float32)
    nc.vector.tensor_copy(
        iota8.rearrange("p (b c) -> p b c", b=2 * NB),
        iota_f[:, None, :].to_broadcast([128, 2 * NB, CR]),
    )
    # pre-warm the exp activation table while the stream runs
    pwz = const.tile([1, 1], dt.float32)
    nc.vector.memset(pwz, 0.0)
    pw = const.tile([1, 1], dt.float32)
    nc.scalar.activation(pw, pwz, Act.Exp)

    b_all_sb = const.tile([CA, 1], dt.float32)
    nc.sync.dma_start(b_all_sb, b_all[:, None])
    b_det_sb = const.tile([C, 1], dt.float32)
    nc.sync.dma_start(b_det_sb, b_det[:, None])

    labrow_i = st.tile([1, C], dt.int32)
    nc.sync.dma_start(labrow_i, lab)
    labrow_f = st.tile([1, C], dt.float32)
    nc.vector.tensor_copy(labrow_f, labrow_i)
    mask_row = st.tile([1, W2], dt.float32)
    nc.vector.tensor_scalar(mask_row[:, 0:C], labrow_f, 1.0, None, Alu.is_equal)
    nc.vector.tensor_copy(mask_row[:, C:W2], mask_row[:, 0:C])

    isw_row = st.tile([1, NS], dt.float32)
    nc.sync.dma_start(isw_row, isw[None, :])
    isw_col = st.tile([128, NB], dt.float32)
    nc.sync.dma_start(isw_col, iswc)
    boxes_nat = st.tile([128, NB, 4], dt.float32)
    nc.sync.dma_start(boxes_nat, boxes)

    # per-roi box areas [128, NB]
    ab_all = st.tile([128, NB], dt.float32)
    abt = stp.tile([128, NB], dt.float32, tag="abt")
    nc.vector.scalar_tensor_tensor(
        abt, boxes_nat[:, :, 2], 1.0, boxes_nat[:, :, 0], Alu.add, Alu.subtract)
    nc.vector.scalar_tensor_tensor(
        ab_all, boxes_nat[:, :, 3], 1.0, boxes_nat[:, :, 1], Alu.add, Alu.subtract)
    nc.vector.tensor_mul(ab_all, ab_all, abt)

    # ---------------- main GEMM phase (bf16, DMA-bound) ----------------
    # scoresA rows: [0:C] cls, [C:C+CR] r1, [C+CR:CA] r2 ; scoresB rows [0:C] det
    scoresA = psc.tile([128, NS], dt.float32)
    scoresB = psc.tile([128, NS], dt.float32)

    for sk in range(SK):
        if sk == 0:
            t_roi, t_frm, t_ctx = nat0
        else:
            ksl = bass.ts(sk, KS)
            t_roi = natp.tile([128, KS, NS], dt.bfloat16, tag="roi")
            nc.sync.dma_start(t_roi, roi[:, ksl, :])
            t_frm = natp.tile([128, KS, NS], dt.bfloat16, tag="frm")
            nc.sync.dma_start(t_frm, frm[:, ksl, :])
            t_ctx = natp.tile([128, KS, NS], dt.bfloat16, tag="ctx")
            nc.sync.dma_start(t_ctx, ctxm[:, ksl, :])
        for j in range(KS):
            k = sk * KS + j
            nc.tensor.matmul(
                scoresA[0:CA, :], w_all_sb[:, k, :], t_roi[:, j, :],
                start=(k == 0), stop=(k == KT - 1),
            )
        for j in range(KS):
            k = sk * KS + j
            nc.tensor.matmul(
                scoresB[0:C, :], wdc_sb[:, k, 0, :], t_frm[:, j, :],
                start=(k == 0), stop=False,
            )
        for j in range(KS):
            k = sk * KS + j
            nc.tensor.matmul(
                scoresB[0:C, :], wdc_sb[:, k, 1, :], t_ctx[:, j, :],
                start=False, stop=(k == KT - 1),
            )

    # ---------------- class-major stats (log-domain argmax) ----------------
    stk = st.tile([CA, NS], dt.float32)
    nc.scalar.activation(stk, scoresA[0:CA, :], Act.Identity, bias=b_all_sb)
    det_sb = st.tile([C, NS], dt.float32)
    nc.scalar.activation(det_sb, scoresB[0:C, :], Act.Identity, bias=b_det_sb)
    psc.release()
    # de-stack r1/r2 to partition base 0 (SBUF->SBUF DMA moves partitions)
    r1_sb = st.tile([CR, NS], dt.float32)
    nc.sync.dma_start(r1_sb, stk[C:C + CR, :])
    r2_sb = st.tile([CR, NS], dt.float32)
    nc.sync.dma_start(r2_sb, stk[C + CR:CA, :])

    zloc = st.tile([C, 1], dt.float32)
    exp_det = st.tile([C, NS], dt.float32)
    nc.scalar.activation(exp_det, det_sb, Act.Exp, accum_out=zloc)
    s1loc = st.tile([C, 1], dt.float32)
    prod_cd = st.tile([C, NS], dt.float32)
    nc.vector.tensor_mul(prod_cd, stk[0:C, :], exp_det)
    nc.vector.reduce_sum(s1loc, prod_cd, axis=AX.X)

    exp_cls = st.tile([C, NS], dt.float32)
    nc.scalar.activation(exp_cls, stk[0:C, :], Act.Exp)
    exp_r1 = st.tile([CR, NS], dt.float32)
    nc.scalar.activation(exp_r1, r1_sb, Act.Exp)

    ps_s1 = pss1.tile([128, NS], dt.float32, tag="mm")
    nc.tensor.matmul(ps_s1[0:1, 0:NS], ones_col[0:C, :], exp_cls,
                     start=True, stop=True)
    ps_s2 = pss1.tile([128, NS], dt.float32, tag="mm")
    nc.tensor.matmul(ps_s2[0:1, 0:NS], ones_col[0:CR, :], exp_r1,
                     start=True, stop=True)
    # trow = ln(isw) - ln(normalizer), per head.  Both normalizers go
    # through ONE Ln activation so the scheduler cannot interleave exp/ln
    # table loads (the combined Ln depends on both exp matmuls).
    ln_isw = st.tile([1, NS], dt.float32)
    nc.scalar.activation(ln_isw, isw_row, Act.Ln)
    lnin = st.tile([1, 2 * NS], dt.float32)
    nc.vector.tensor_copy(lnin[:, 0:NS], ps_s1[0:1, 0:NS])
    nc.vector.tensor_copy(lnin[:, NS:2 * NS], ps_s2[0:1, 0:NS])
    lnout = st.tile([1, 2 * NS], dt.float32)
    nc.scalar.activation(lnout, lnin, Act.Ln)
    trow1 = st.tile([1, NS], dt.float32)
    nc.vector.tensor_sub(trow1, ln_isw, lnout[:, 0:NS])
    trow2 = st.tile([1, NS], dt.float32)
    nc.vector.tensor_sub(trow2, ln_isw, lnout[:, NS:2 * NS])

    ps_b1 = pss1.tile([128, NS], dt.float32, tag="mm")
    nc.tensor.matmul(ps_b1[0:C, 0:NS], ones_row[:, 0:C], trow1,
                     start=True, stop=True)
    ps_b2 = pss1.tile([128, NS], dt.float32, tag="mm")
    nc.tensor.matmul(ps_b2[0:CR, 0:NS], ones_row[:, 0:CR], trow2,
                     start=True, stop=True)

    # lp1 = cls + det + trow1 ; lq2 = r1 + trow2  (argmax-equivalent logs)
    lp1 = st.tile([C, NS], dt.float32)
    nc.vector.tensor_add(lp1, stk[0:C, :], det_sb)
    nc.vector.tensor_add(lp1, lp1, ps_b1[0:C, 0:NS])
    lq2 = st.tile([CR, NS], dt.float32)
    nc.vector.tensor_add(lq2, r1_sb, ps_b2[0:CR, 0:NS])

    vm1 = st.tile([C, 1], dt.float32)
    nc.vector.reduce_max(vm1, lp1, axis=AX.X)
    vm2 = st.tile([CR, 1], dt.float32)
    nc.vector.reduce_max(vm2, lq2, axis=AX.X)

    # local per-class argmax boxes via sel-mask matmuls (exact one-hot gather)
    sel1 = st.tile([C, NS], dt.float32)
    nc.vector.tensor_scalar(sel1, lp1, vm1, None, Alu.is_equal)
    sel2 = st.tile([CR, NS], dt.float32)
    nc.vector.tensor_scalar(sel2, lq2, vm2, None, Alu.is_equal)
    psq = pss2.tile([128, 64], dt.float32, tag="acc")
    for b in range(NB):
        bsl = bass.ts(b, 128)
        ptx = pst.tile([128, 64], dt.float32, tag="pt")
        nc.tensor.transpose(ptx[:, 0:C], sel1[:, bsl], ident[0:C, 0:C])
        nc.tensor.transpose(ptx[:, 32:32 + CR], sel2[:, bsl], ident[0:CR, 0:CR])
        sT = stp.tile([128, 64], dt.float32, tag="sT")
        nc.vector.tensor_copy(sT[:, 0:C], ptx[:, 0:C])
        nc.vector.tensor_copy(sT[:, 32:32 + CR], ptx[:, 32:32 + CR])
        nc.tensor.matmul(
            psq[0:4, 0:C], boxes_nat[:, b, :], sT[:, 0:C],
            start=(b == 0), stop=(b == NB - 1), skip_group_check=True,
        )
        nc.tensor.matmul(
            psq[0:4, C:W2], boxes_nat[:, b, :], sT[:, 33:33 + C],
            start=(b == 0), stop=(b == NB - 1), skip_group_check=True,
        )
    bc_sb = st.tile([4, W2], dt.float32)
    nc.vector.tensor_copy(bc_sb, psq[0:4, 0:W2])

    # ---------------- G1: AllGather of all cross-core state ----------------
    g1_in = dp.tile([GW], dt.float32)
    g1_out = dp.tile([n_cores * GW], dt.float32)
    nc.sync.dma_start(g1_in[0:C], vm1[:, 0])
    nc.sync.dma_start(g1_in[C:C + CR], vm2[:, 0])
    nc.sync.dma_start(g1_in[41:201], bc_sb)
    nc.sync.dma_start(g1_in[201:221], zloc[:, 0])
    nc.sync.dma_start(g1_in[221:241], s1loc[:, 0])
    nc.gpsimd.collective_compute(
        "AllGather", Alu.bypass, replica_groups=group,
        ins=[g1_in.opt()], outs=[g1_out.opt()],
    )

    # ---- collective-independent prep, emitted here to fill G1 latency ----
    # refine-head scores to roi-major [128, (b,s), CR], then log-softmax
    rts = st.tile([128, NB * 2 * CR], dt.float32)
    for b in range(NB):
        bsl = bass.ts(b, 128)
        ptr = pst.tile([128, 64], dt.float32, tag="pt")
        nc.tensor.transpose(ptr[:, 0:CR], r1_sb[:, bsl], ident[0:CR, 0:CR])
        nc.tensor.transpose(ptr[:, CR:2 * CR], r2_sb[:, bsl], ident[0:CR, 0:CR])
        nc.vector.tensor_copy(rts[:, b * 2 * CR:(b + 1) * 2 * CR], ptr[:, 0:2 * CR])
    rts3 = rts.rearrange("p (g c) -> p g c", g=2 * NB)
    rmax = st.tile([128, 2 * NB], dt.float32)
    nc.vector.reduce_max(rmax, rts3, axis=AX.X)
    xs_all = st.tile([128, NB * 2 * CR], dt.float32)
    xs3 = xs_all.rearrange("p (g c) -> p g c", g=2 * NB)
    nc.vector.tensor_tensor(
        xs3, rts3, rmax[:, :, None].to_broadcast([128, 2 * NB, CR]), Alu.subtract)
    ex_all = st.tile([128, NB * 2 * CR], dt.float32)
    nc.scalar.activation(ex_all, xs_all, Act.Exp)
    ssum = st.tile([128, 2 * NB], dt.float32)
    nc.vector.reduce_sum(ssum, ex_all.rearrange("p (g c) -> p g c", g=2 * NB),
                         axis=AX.X)
    lse = st.tile([128, 2 * NB], dt.float32)
    nc.scalar.activation(lse, ssum, Act.Ln)
    nc.vector.tensor_tensor(
        xs3, xs3, lse[:, :, None].to_broadcast([128, 2 * NB, CR]), Alu.subtract)
    # xs_all now holds log-probs for both refine heads

    # ---------------- G1 readback + cross-core combine ----------------
    g_sb = st.tile([n_cores, GW], dt.float32)
    nc.sync.dma_start(g_sb, g1_out.rearrange("(r w) -> r w", r=n_cores))
    vmx = st.tile([n_cores, 41], dt.float32)
    nc.gpsimd.partition_all_reduce(
        vmx, g_sb[:, 0:41], channels=n_cores, reduce_op=bass_isa.ReduceOp.max
    )
    selc = st.tile([n_cores, 41], dt.float32)
    nc.vector.tensor_tensor(selc, g_sb[:, 0:41], vmx, Alu.is_equal)
    masked = st.tile([n_cores, 160], dt.float32)
    mview = masked.rearrange("p (co s c) -> p co s c", co=4, s=2)
    gview = g_sb[:, 41:201].rearrange("p (co s c) -> p co s c", co=4, s=2)
    nc.vector.tensor_tensor(
        mview[:, :, 0, :], gview[:, :, 0, :],
        selc[:, None, 0:C].to_broadcast([n_cores, 4, C]), Alu.mult,
    )
    nc.vector.tensor_tensor(
        mview[:, :, 1, :], gview[:, :, 1, :],
        selc[:, None, CR:CR + C].to_broadcast([n_cores, 4, C]), Alu.mult,
    )
    ps_qr = pss1.tile([128, NS], dt.float32, tag="mm")
    nc.tensor.matmul(ps_qr[0:1, 0:160], ones_col[0:n_cores, :], masked,
                     start=True, stop=True, skip_group_check=True)
    nc.tensor.matmul(ps_qr[0:1, 160:200], ones_col[0:n_cores, :], g_sb[:, 201:241],
                     start=True, stop=True, skip_group_check=True)
    qzs = st.tile([1, 200], dt.float32)
    nc.vector.tensor_copy(qzs, ps_qr[0:1, 0:200])

    # broadcast [boxes(160) | mask(40)] to all 128 partitions
    ps_q = pss1.tile([128, NS], dt.float32, tag="mm")
    nc.tensor.matmul(ps_q[:, 0:160], ones_row[0:1, :], qzs[:, 0:160],
                     start=True, stop=True, skip_group_check=True)
    nc.tensor.matmul(ps_q[:, 160:200], ones_row[0:1, :], mask_row,
                     start=True, stop=True, skip_group_check=True)
    # materialize NB-tiled copy so later ops use plain strided in0 views
    Q4 = st.tile([128, NB, 200], dt.float32)
    nc.vector.tensor_copy(
        Q4, ps_q[:, None, 0:200].to_broadcast([128, NB, 200]))

    # query areas + roi areas [128, NB, 40]
    aqt = stp.tile([128, NB, W2], dt.float32, tag="aqt")
    nc.vector.scalar_tensor_tensor(
        aqt, Q4[:, :, 80:120], 1.0, Q4[:, :, 0:40], Alu.add, Alu.subtract)
    ab40 = st.tile([128, NB, W2], dt.float32)
    nc.vector.scalar_tensor_tensor(
        ab40, Q4[:, :, 120:160], 1.0, Q4[:, :, 40:80], Alu.add, Alu.subtract)
    nc.vector.tensor_mul(ab40, ab40, aqt)
    nc.vector.tensor_tensor(
        ab40, ab40, ab_all[:, :, None].to_broadcast([128, NB, W2]), Alu.add)

    # ---------------- batched paired IoU / assignment / loss ----------------
    xi1 = stp.tile([128, NB, W2], dt.float32, tag="xi1")
    nc.vector.tensor_tensor(
        xi1, Q4[:, :, 0:40],
        boxes_nat[:, :, 0:1].to_broadcast([128, NB, W2]), Alu.max)
    yi1 = stp.tile([128, NB, W2], dt.float32, tag="yi1")
    nc.vector.tensor_tensor(
        yi1, Q4[:, :, 40:80],
        boxes_nat[:, :, 1:2].to_broadcast([128, NB, W2]), Alu.max)
    xi2 = stp.tile([128, NB, W2], dt.float32, tag="xi2")
    nc.vector.tensor_tensor(
        xi2, Q4[:, :, 80:120],
        boxes_nat[:, :, 2:3].to_broadcast([128, NB, W2]), Alu.min)
    yi2 = stp.tile([128, NB, W2], dt.float32, tag="yi2")
    nc.vector.tensor_tensor(
        yi2, Q4[:, :, 120:160],
        boxes_nat[:, :, 3:4].to_broadcast([128, NB, W2]), Alu.min)
    nc.vector.scalar_tensor_tensor(xi2, xi2, 1.0, xi1, Alu.add, Alu.subtract)
    nc.vector.tensor_scalar_max(xi2, xi2, 0.0)   # iw
    nc.vector.scalar_tensor_tensor(yi2, yi2, 1.0, yi1, Alu.add, Alu.subtract)
    nc.vector.tensor_scalar_max(yi2, yi2, 0.0)   # ih
    inter = stp.tile([128, NB, W2], dt.float32, tag="inter")
    nc.vector.tensor_mul(inter, xi2, yi2)
    un = stp.tile([128, NB, W2], dt.float32, tag="un")
    nc.vector.tensor_sub(un, ab40, inter)
    unf = un.rearrange("p b w -> p (b w)")
    nc.vector.reciprocal(unf, unf)
    ov = stp.tile([128, NB, W2], dt.float32, tag="ov")
    nc.vector.tensor_mul(ov, inter, un)
    # mask image-level negatives to exactly -1: ov = (ov+1)*mask - 1
    nc.vector.scalar_tensor_tensor(
        ov, ov, 1.0, Q4[:, :, 160:200], Alu.add, Alu.mult)
    ovf = ov.rearrange("p b w -> p (b w)")
    nc.vector.tensor_scalar_add(ovf, ovf, -1.0)

    ov4 = ov.rearrange("p b (s c) -> p (b s) c", s=2)
    mo = stp.tile([128, 2 * NB], dt.float32, tag="mo")
    nc.vector.reduce_max(mo, ov4, axis=AX.X)
    meq = stp.tile([128, 2 * NB, C], dt.float32, tag="meq")
    nc.vector.tensor_tensor(
        meq, ov4, mo[:, :, None].to_broadcast([128, 2 * NB, C]), Alu.is_equal)
    nc.vector.tensor_tensor(
        meq, meq, iota_m1k[:, None, :].to_broadcast([128, 2 * NB, C]), Alu.mult)
    gt = stp.tile([128, 2 * NB], dt.float32, tag="gt")
    nc.vector.tensor_reduce(gt, meq, axis=AX.X, op=Alu.min)

    fg = stp.tile([128, 2 * NB], dt.float32, tag="fg")
    nc.vector.tensor_scalar(fg, mo, 0.5, None, Alu.is_gt)
    bgt = stp.tile([128, 2 * NB], dt.float32, tag="bgt")
    nc.vector.tensor_scalar(bgt, mo, 0.5, None, Alu.is_lt)
    bg = stp.tile([128, 2 * NB], dt.float32, tag="bg")
    nc.vector.scalar_tensor_tensor(bg, mo, 0.1, bgt, Alu.is_ge, Alu.mult)
    keep = stp.tile([128, 2 * NB], dt.float32, tag="keep")
    nc.vector.tensor_add(keep, fg, bg)
    col = stp.tile([128, 2 * NB], dt.float32, tag="col")
    nc.vector.scalar_tensor_tensor(col, gt, 1001.0, fg, Alu.add, Alu.mult)

    oh = stp.tile([128, 2 * NB, CR], dt.float32, tag="oh")
    nc.vector.tensor_tensor(
        oh, iota8.rearrange("p (g c) -> p g c", g=2 * NB),
        col[:, :, None].to_broadcast([128, 2 * NB, CR]), Alu.is_equal)
    nc.vector.tensor_tensor(oh, oh, xs3, Alu.mult)
    lpsel = stp.tile([128, 2 * NB], dt.float32, tag="lpsel")
    nc.vector.reduce_sum(lpsel, oh, axis=AX.X)

    wk = st.tile([128, 16], dt.float32)
    wl3 = wk[:, 0:8].rearrange("p (b s) -> p b s", s=2)
    nc.vector.tensor_tensor(
        wl3, keep.rearrange("p (b s) -> p b s", s=2),
        isw_col[:, :, None].to_broadcast([128, NB, 2]), Alu.mult)
    nc.vector.tensor_tensor(wk[:, 0:8], wk[:, 0:8], lpsel, Alu.mult)
    nc.vector.tensor_copy(wk[:, 8:16], keep)
    ps_l = pss2.tile([128, 64], dt.float32, tag="acc")
    nc.tensor.matmul(ps_l[0:16, 0:1], wk, ones_col, start=True, stop=True)
    l16 = st.tile([16, 1], dt.float32)
    nc.vector.tensor_copy(l16, ps_l[0:16, 0:1])

    # ---------------- R3: AllReduce(add) of loss partials ----------------
    cc3_in = dp.tile([16], dt.float32)
    cc3_out = dp.tile([16], dt.float32)
    nc.sync.dma_start(cc3_in, l16[:, 0])
    nc.gpsimd.collective_compute(
        "AllReduce", Alu.add, replica_groups=group,
        ins=[cc3_in.opt()], outs=[cc3_out.opt()],
    )

    # ---- hinge loss from qzs (fills R3 latency) ----
    zrow = qzs[:, 160:180]
    s1row = qzs[:, 180:200]
    zinv = st.tile([1, C], dt.float32)
    nc.vector.reciprocal(zinv, zrow)
    dcs = st.tile([1, C], dt.float32)
    nc.vector.tensor_mul(dcs, s1row, zinv)
    hv = st.tile([1, C], dt.float32)
    nc.vector.tensor_mul(hv, labrow_f, dcs)
    nc.scalar.activation(hv, hv, Act.Relu, bias=1.0, scale=-1.0)  # relu(1-lab*dcs)
    h = st.tile([1, 1], dt.float32)
    nc.vector.reduce_sum(h, hv, axis=AX.X)
    nc.scalar.mul(h, h, 1.0 / C)

    # ---------------- R3 readback + final scalar ----------------
    l4 = st.tile([1, 16], dt.float32)
    nc.sync.dma_start(l4, cc3_out[None, :])
    nums = st.tile([1, 2], dt.float32)
    nc.vector.reduce_sum(
        nums, l4[:, 0:8].rearrange("a (b s) -> a s b", s=2), axis=AX.X)
    dens = st.tile([1, 2], dt.float32)
    nc.vector.reduce_sum(
        dens, l4[:, 8:16].rearrange("a (b s) -> a s b", s=2), axis=AX.X)
    dinv = st.tile([1, 2], dt.float32)
    nc.vector.reciprocal(dinv, dens)
    rl = st.tile([1, 2], dt.float32)
    nc.vector.tensor_mul(rl, nums, dinv)
    rsum = st.tile([1, 1], dt.float32)
    nc.vector.reduce_sum(rsum, rl, axis=AX.X)
    tot = st.tile([1, 1], dt.float32)
    nc.scalar.mul(tot, rsum, -0.1)
    nc.vector.tensor_add(tot, tot, h)
    nc.sync.dma_start(loss, tot)

    for pool in (dp, pss2, pss1, pst, natp, stp, st, const):
        pool.release()


def build_program(NS=512, F=4096, n_cores=8):
    nc = bacc.Bacc(
        "TRN2", target_bir_lowering=False, debug=False, num_devices=n_cores
    )
    KT = F // 128
    NB = NS // 128
    bf = dt.bfloat16
    f32 = dt.float32
    roi = nc.dram_tensor("roi", [128, KT, NS], bf, kind="ExternalInput").ap()
    frm = nc.dram_tensor("frm", [128, KT, NS], bf, kind="ExternalInput").ap()
    ctxm = nc.dram_tensor("ctxm", [128, KT, NS], bf, kind="ExternalInput").ap()
    w_all = nc.dram_tensor("w_all", [128, KT, CA], bf, kind="ExternalInput").ap()
    wdc = nc.dram_tensor("wdc", [128, KT, 2, C], bf, kind="ExternalInput").ap()
    b_all = nc.dram_tensor("b_all", [CA], f32, kind="ExternalInput").ap()
    b_det = nc.dram_tensor("b_det", [C], f32, kind="ExternalInput").ap()
    boxes = nc.dram_tensor("boxes", [128, NB, 4], f32, kind="ExternalInput").ap()
    isw = nc.dram_tensor("isw", [NS], f32, kind="ExternalInput").ap()
    iswc = nc.dram_tensor("iswc", [128, NB], f32, kind="ExternalInput").ap()
    lab = nc.dram_tensor("lab", [1, C], dt.int32, kind="ExternalInput").ap()
    loss = nc.dram_tensor("loss", [1, 1], f32, kind="ExternalOutput").ap()
    aps = (roi, frm, ctxm, w_all, wdc, b_all, b_det, boxes, isw, iswc, lab,
           loss)
    with tile.TileContext(nc) as tc:
        _emit(nc, tc, aps, NS, F, n_cores)
    nc.compile()
    return nc


def make_in_maps(inputs, NS, n_cores):
    f32 = np.float32
    bf = ml_dtypes.bfloat16
    F = inputs["fc7_roi"].shape[1]
    KT = F // 128
    NB = NS // 128

    def _wpack(w):
        # [F, cols] -> [128, KT, cols] contiguous
        return np.ascontiguousarray(
            np.asarray(w, f32).reshape(KT, 128, -1).transpose(1, 0, 2)
        ).astype(bf)

    w_all = _wpack(np.concatenate(
        [np.asarray(inputs["W_cls"]), np.asarray(inputs["W_r1"]),
         np.asarray(inputs["W_r2"])], axis=1))
    wd = np.asarray(inputs["W_det"], f32)
    wdc = np.ascontiguousarray(
        np.stack([wd.reshape(KT, 128, C), -wd.reshape(KT, 128, C)], axis=2)
        .transpose(1, 0, 2, 3)).astype(bf)
    b_all = np.ascontiguousarray(
        np.concatenate([np.asarray(inputs["b_cls"]), np.asarray(inputs["b_r1"]),
                        np.asarray(inputs["b_r2"])]), f32)
    b_det = np.ascontiguousarray(np.asarray(inputs["b_det"]), f32)
    boxes = np.asarray(inputs["ss_boxes"], f32)[:, 1:5]
    isw = np.ascontiguousarray(np.asarray(inputs["IS_weight"])[:, 0], f32)
    lab = np.ascontiguousarray(np.asarray(inputs["image_level_label"]), np.int32)
    roi = np.asarray(inputs["fc7_roi"], f32).T.astype(bf)
    frm = np.asarray(inputs["fc7_frame"], f32).T.astype(bf)
    ctxm = np.asarray(inputs["fc7_context"], f32).T.astype(bf)

    def _pack(a, sl):
        # [F, NS] slice -> [128, KT, NS]: 8KB-contiguous per-partition runs
        return np.ascontiguousarray(
            a[:, sl].reshape(KT, 128, -1).transpose(1, 0, 2))

    in_maps = []
    for c in range(n_cores):
        sl = slice(c * NS, (c + 1) * NS)
        bsl = boxes[sl]
        isl = isw[sl]
        in_maps.append({
            "roi": _pack(roi, sl),
            "frm": _pack(frm, sl),
            "ctxm": _pack(ctxm, sl),
            "w_all": w_all, "wdc": wdc, "b_all": b_all, "b_det": b_det,
            "boxes": np.ascontiguousarray(
                bsl.reshape(NB, 128, 4).transpose(1, 0, 2)),
            "isw": isl,
            "iswc": np.ascontiguousarray(isl.reshape(NB, 128).T),
            "lab": lab,
        })
    return in_maps


_PROG_CACHE = {}


def _get_prog(NS, F, n_cores):
    key = (NS, F, n_cores)
    if key not in _PROG_CACHE:
        _PROG_CACHE[key] = build_program(NS, F, n_cores)
    return _PROG_CACHE[key]


def kernel(**inputs):
    n_cores = 8
    N, F = inputs["fc7_roi"].shape
    NS = N // n_cores
    prog = _get_prog(NS, F, n_cores)
    in_maps = make_in_maps(inputs, NS, n_cores)
    res = run_bass_kernel_spmd(prog, in_maps, list(range(n_cores))).results
    return np.float32(res[0]["loss"].reshape(()))


# revision 28
# speedup vs baseline: 1.6211x; 1.6211x over previous
# Trainium2 Bass kernel for nn_Network_515396076038 (nms_detection / OICR-style loss).
#
# Strategy (8 NeuronCores, data-parallel over the N=4096 proposals):
#   - Each core owns NS = N/8 = 512 rois and streams its shard of fc7_roi /
#     fc7_frame / fc7_context from HBM in bf16 (12 MB per core -> memory
#     roofline ~34us), computing the GEMM heads in class-major layout [C, NS]
#     (scores^T = W^T @ X^T accumulated over F).  det = W@frm - W@ctx is
#     computed on the PE by accumulating ctx with -W (host-negated weights),
#     so the vector engine does no work during the stream.
#   - Per-class argmax over rois is computed in LOG domain:
#     log p = cls + det + ln(isw) - ln(sum_c exp(cls)) -- the global det
#     softmax normalizer is a per-class constant and cannot change the
#     argmax, and the log form avoids expensive vector reciprocals.
#   - One AllGather ships each core's per-class maxima + candidate boxes
#     (sel-mask matmul gather) + softmax partial sums; refine-head
#     log-softmax prep fills the collective latency.  A final tiny
#     AllReduce(add) sums 16 loss partials (4 roi-blocks x 2 supervisions x
#     {num, den}); the per-block sums ride the collective for free.
#   - IoU / fg-bg assignment / one-hot log-prob gather run roi-major,
#     batched across all 4 roi-blocks and both supervisions in single
#     vector ops on [128, 160] tiles via stride-0 broadcast views.
import sys

for _p in ("/opt/trn_rl_repo",):
    if _p not in sys.path:
        sys.path.append(_p)

import numpy as np
import ml_dtypes

import concourse.bass as bass
import concourse.bass_isa as bass_isa
import concourse.mybir as mybir
import concourse.tile as tile
from concourse import bacc
from concourse.bass_utils import run_bass_kernel_spmd
from concourse.masks import make_identity

dt = mybir.dt
Alu = mybir.AluOpType
Act = mybir.ActivationFunctionType
AX = mybir.AxisListType

C = 20      # foreground classes
CR = C + 1  # refine head classes (background + C)
CA = C + 2 * CR  # stacked roi-head outputs: cls | r1 | r2 = 62
W2 = 2 * C


def _emit(nc, tc, aps, NS, F, n_cores):
    NB = NS // 128
    KT = F // 128
    group = [list(range(n_cores))]
    GW = 241  # AllGather row: vm1[20] vm2[21] boxes[160] z[20] s1[20]

    (roi, frm, ctxm, w_all, wdc, b_all, b_det, boxes, isw, iswc, lab,
     loss) = aps

    const = tc.alloc_tile_pool(name="const", bufs=1)
    st = tc.alloc_tile_pool(name="st", bufs=1)
    stp = tc.alloc_tile_pool(name="stp", bufs=2)
    natp = tc.alloc_tile_pool(name="natp", bufs=2)
    pst = tc.alloc_tile_pool(name="pst", bufs=2, space="PSUM")
    pss1 = tc.alloc_tile_pool(name="pss1", bufs=2, space="PSUM")
    pss2 = tc.alloc_tile_pool(name="pss2", bufs=1, space="PSUM")
    dp = tc.alloc_tile_pool(name="dp", bufs=1, space="DRAM")
    # psc is created last: it is the first pool released (LIFO pool stack)
    psc = tc.alloc_tile_pool(name="psc", bufs=1, space="PSUM")

    # ---- dummy collective: absorbs CC-engine warmup under the DMA stream
    dz = const.tile([1, 1], dt.float32)
    nc.vector.memset(dz, 0.0)
    cc0_in = dp.tile([4], dt.float32)
    cc0_out = dp.tile([4 * n_cores], dt.float32)
    nc.sync.dma_start(cc0_in[0:1], dz[:, 0])
    nc.sync.dma_start(cc0_in[1:2], dz[:, 0])
    nc.sync.dma_start(cc0_in[2:3], dz[:, 0])
    nc.sync.dma_start(cc0_in[3:4], dz[:, 0])
    nc.gpsimd.collective_compute(
        "AllGather", Alu.bypass, replica_groups=group,
        ins=[cc0_in.opt()], outs=[cc0_out.opt()],
    )

    # ---- big stream DMAs first: weights + first super-tile ----
    KS = min(8, KT)
    SK = KT // KS
    w_all_sb = const.tile([128, KT, CA], dt.bfloat16)
    nc.sync.dma_start(w_all_sb, w_all)
    wdc_sb = const.tile([128, KT, 2, C], dt.bfloat16)
    nc.sync.dma_start(wdc_sb, wdc)
    nat0 = []
    for nm, srcp in (("roi", roi), ("frm", frm), ("ctx", ctxm)):
        t = natp.tile([128, KS, NS], dt.bfloat16, tag=nm)
        nc.sync.dma_start(t, srcp[:, bass.ts(0, KS), :])
        nat0.append(t)

    # ---------------- constants / setup (overlaps the GEMM stream) --------
    ident = const.tile([128, 128], dt.float32)
    make_identity(nc, ident)
    ones_col = const.tile([128, 1], dt.float32)
    nc.vector.memset(ones_col, 1.0)
    ones_row = const.tile([1, 128], dt.float32)
    nc.vector.memset(ones_row, 1.0)
    iota_i = const.tile([128, CR], dt.int32)
    nc.gpsimd.iota(iota_i, pattern=[[1, CR]], base=0, channel_multiplier=0)
    iota_f = const.tile([128, CR], dt.float32)
    nc.vector.tensor_copy(iota_f, iota_i)
    iota_m1k = const.tile([128, C], dt.float32)
    nc.vector.tensor_scalar_add(iota_m1k, iota_f[:, :C], -1000.0)
    iota8 = const.tile([128, 2 * NB * CR], dt.float32)
    nc.vector.tensor_copy(
        iota8.rearrange("p (b c) -> p b c", b=2 * NB),
        iota_f[:, None, :].to_broadcast([128, 2 * NB, CR]),
    )
    # pre-warm the exp activation table while the stream runs
    pwz = const.tile([1, 1], dt.float32)
    nc.vector.memset(pwz, 0.0)
    pw = const.tile([1, 1], dt.float32)
    nc.scalar.activation(pw, pwz, Act.Exp)

    b_all_sb = const.tile([CA, 1], dt.float32)
    nc.sync.dma_start(b_all_sb, b_all[:, None])
    b_det_sb = const.tile([C, 1], dt.float32)
    nc.sync.dma_start(b_det_sb, b_det[:, None])

    labrow_i = st.tile([1, C], dt.int32)
    nc.sync.dma_start(labrow_i, lab)
    labrow_f = st.tile([1, C], dt.float32)
    nc.vector.tensor_copy(labrow_f, labrow_i)
    mask_row = st.tile([1, W2], dt.float32)
    nc.vector.tensor_scalar(mask_row[:, 0:C], labrow_f, 1.0, None, Alu.is_equal)
    nc.vector.tensor_copy(mask_row[:, C:W2], mask_row[:, 0:C])

    isw_row = st.tile([1, NS], dt.float32)
    nc.sync.dma_start(isw_row, isw[None, :])
    isw_col = st.tile([128, NB], dt.float32)
    nc.sync.dma_start(isw_col, iswc)
    boxes_nat = st.tile([128, NB, 4], dt.float32)
    nc.sync.dma_start(boxes_nat, boxes)

    # per-roi box areas [128, NB]
    ab_all = st.tile([128, NB], dt.float32)
    abt = stp.tile([128, NB], dt.float32, tag="abt")
    nc.vector.scalar_tensor_tensor(
        abt, boxes_nat[:, :, 2], 1.0, boxes_nat[:, :, 0], Alu.add, Alu.subtract)
    nc.vector.scalar_tensor_tensor(
        ab_all, boxes_nat[:, :, 3], 1.0, boxes_nat[:, :, 1], Alu.add, Alu.subtract)
    nc.vector.tensor_mul(ab_all, ab_all, abt)

    # ---------------- main GEMM phase (bf16, DMA-bound) ----------------
    # scoresA rows: [0:C] cls, [C:C+CR] r1, [C+CR:CA] r2 ; scoresB rows [0:C] det
    scoresA = psc.tile([128, NS], dt.float32)
    scoresB = psc.tile([128, NS], dt.float32)

    for sk in range(SK):
        if sk == 0:
            t_roi, t_frm, t_ctx = nat0
        else:
            ksl = bass.ts(sk, KS)
            t_roi = natp.tile([128, KS, NS], dt.bfloat16, tag="roi")
            nc.sync.dma_start(t_roi, roi[:, ksl, :])
            t_frm = natp.tile([128, KS, NS], dt.bfloat16, tag="frm")
            nc.sync.dma_start(t_frm, frm[:, ksl, :])
            t_ctx = natp.tile([128, KS, NS], dt.bfloat16, tag="ctx")
            nc.sync.dma_start(t_ctx, ctxm[:, ksl, :])
        for j in range(KS):
            k = sk * KS + j
            nc.tensor.matmul(
                scoresA[0:CA, :], w_all_sb[:, k, :], t_roi[:, j, :],
                start=(k == 0), stop=(k == KT - 1),
            )
        for j in range(KS):
            k = sk * KS + j
            nc.tensor.matmul(
                scoresB[0:C, :], wdc_sb[:, k, 0, :], t_frm[:, j, :],
                start=(k == 0), stop=False,
            )
        for j in range(KS):
            k = sk * KS + j
            nc.tensor.matmul(
                scoresB[0:C, :], wdc_sb[:, k, 1, :], t_ctx[:, j, :],
                start=False, stop=(k == KT - 1),
            )

    # ---------------- class-major stats (log-domain argmax) ----------------
    stk = st.tile([CA, NS], dt.float32)
    nc.scalar.activation(stk, scoresA[0:CA, :], Act.Identity, bias=b_all_sb)
    det_sb = st.tile([C, NS], dt.float32)
    nc.scalar.activation(det_sb, scoresB[0:C, :], Act.Identity, bias=b_det_sb)
    psc.release()
    # de-stack r1/r2 to partition base 0 (SBUF->SBUF DMA moves partitions)
    r1_sb = st.tile([CR, NS], dt.float32)
    nc.sync.dma_start(r1_sb, stk[C:C + CR, :])
    r2_sb = st.tile([CR, NS], dt.float32)
    nc.sync.dma_start(r2_sb, stk[C + CR:CA, :])

    # vmzs cols: 0 = per-class max (filled later), 1 = z partial, 2 = s1
    vmzs = st.tile([C, 3], dt.float32)
    exp_det = st.tile([C, NS], dt.float32)
    nc.scalar.activation(exp_det, det_sb, Act.Exp, accum_out=vmzs[:, 1:2])
    prod_cd = st.tile([C, NS], dt.float32)
    nc.vector.tensor_mul(prod_cd, stk[0:C, :], exp_det)
    nc.vector.reduce_sum(vmzs[:, 2:3], prod_cd, axis=AX.X)

    exp_cls = st.tile([C, NS], dt.float32)
    nc.scalar.activation(exp_cls, stk[0:C, :], Act.Exp)
    exp_r1 = st.tile([CR, NS], dt.float32)
    nc.scalar.activation(exp_r1, r1_sb, Act.Exp)

    ps_s1 = pss1.tile([128, NS], dt.float32, tag="mm")
    nc.tensor.matmul(ps_s1[0:1, 0:NS], ones_col[0:C, :], exp_cls,
                     start=True, stop=True)
    ps_s2 = pss1.tile([128, NS], dt.float32, tag="mm")
    nc.tensor.matmul(ps_s2[0:1, 0:NS], ones_col[0:CR, :], exp_r1,
                     start=True, stop=True)
    # trow = ln(isw) - ln(normalizer), per head.  Both normalizers go
    # through ONE Ln activation so the scheduler cannot interleave exp/ln
    # table loads (the combined Ln depends on both exp matmuls).
    ln_isw = st.tile([1, NS], dt.float32)
    nc.scalar.activation(ln_isw, isw_row, Act.Ln)
    lnin = st.tile([1, 2 * NS], dt.float32)
    nc.vector.tensor_copy(lnin[:, 0:NS], ps_s1[0:1, 0:NS])
    nc.vector.tensor_copy(lnin[:, NS:2 * NS], ps_s2[0:1, 0:NS])
    lnout = st.tile([1, 2 * NS], dt.float32)
    nc.scalar.activation(lnout, lnin, Act.Ln)
    trow1 = st.tile([1, NS], dt.float32)
    nc.vector.tensor_sub(trow1, ln_isw, lnout[:, 0:NS])
    trow2 = st.tile([1, NS], dt.float32)
    nc.vector.tensor_sub(trow2, ln_isw, lnout[:, NS:2 * NS])

    ps_b1 = pss1.tile([128, NS], dt.float32, tag="mm")
    nc.tensor.matmul(ps_b1[0:C, 0:NS], ones_row[:, 0:C], trow1,
                     start=True, stop=True)
    ps_b2 = pss1.tile([128, NS], dt.float32, tag="mm")
    nc.tensor.matmul(ps_b2[0:CR, 0:NS], ones_row[:, 0:CR], trow2,
                     start=True, stop=True)

    # lp1 = cls + det + trow1 ; lq2 = r1 + trow2  (argmax-equivalent logs)
    lp1 = st.tile([C, NS], dt.float32)
    nc.vector.tensor_add(lp1, stk[0:C, :], det_sb)
    nc.vector.tensor_add(lp1, lp1, ps_b1[0:C, 0:NS])
    lq2 = st.tile([CR, NS], dt.float32)
    nc.vector.tensor_add(lq2, r1_sb, ps_b2[0:CR, 0:NS])

    nc.vector.reduce_max(vmzs[:, 0:1], lp1, axis=AX.X)
    vm1 = vmzs[:, 0:1]
    vm2 = st.tile([CR, 1], dt.float32)
    nc.vector.reduce_max(vm2, lq2, axis=AX.X)

    # local per-class argmax boxes via sel-mask matmuls (exact one-hot gather)
    sel1 = st.tile([C, NS], dt.float32)
    nc.vector.tensor_scalar(sel1, lp1, vmzs[:, 0:1], None, Alu.is_equal)
    sel2 = st.tile([CR, NS], dt.float32)
    nc.vector.tensor_scalar(sel2, lq2, vm2, None, Alu.is_equal)
    psq = pss2.tile([128, 64], dt.float32, tag="acc")
    for b in range(NB):
        bsl = bass.ts(b, 128)
        ptx = pst.tile([128, 64], dt.float32, tag="pt")
        nc.tensor.transpose(ptx[:, 0:C], sel1[:, bsl], ident[0:C, 0:C])
        nc.tensor.transpose(ptx[:, 32:32 + CR], sel2[:, bsl], ident[0:CR, 0:CR])
        sT = stp.tile([128, 64], dt.float32, tag="sT")
        nc.vector.tensor_copy(sT[:, 0:C], ptx[:, 0:C])
        nc.vector.tensor_copy(sT[:, 32:32 + CR], ptx[:, 32:32 + CR])
        nc.tensor.matmul(
            psq[0:4, 0:C], boxes_nat[:, b, :], sT[:, 0:C],
            start=(b == 0), stop=(b == NB - 1), skip_group_check=True,
        )
        nc.tensor.matmul(
            psq[0:4, C:W2], boxes_nat[:, b, :], sT[:, 33:33 + C],
            start=(b == 0), stop=(b == NB - 1), skip_group_check=True,
        )
    bc_sb = st.tile([4, W2], dt.float32)
    nc.vector.tensor_copy(bc_sb, psq[0:4, 0:W2])

    # ---------------- G1: AllGather of all cross-core state ----------------
    g1_in = dp.tile([GW], dt.float32)
    g1_out = dp.tile([n_cores * GW], dt.float32)
    nc.sync.dma_start(g1_in[0:60], vmzs)
    nc.sync.dma_start(g1_in[60:81], vm2[:, 0])
    nc.sync.dma_start(g1_in[81:241], bc_sb)
    nc.gpsimd.collective_compute(
        "AllGather", Alu.bypass, replica_groups=group,
        ins=[g1_in.opt()], outs=[g1_out.opt()],
    )

    # ---- collective-independent prep, emitted here to fill G1 latency ----
    # refine-head scores to roi-major [128, (b,s), CR], then log-softmax
    rts = st.tile([128, NB * 2 * CR], dt.float32)
    for b in range(NB):
        bsl = bass.ts(b, 128)
        ptr = pst.tile([128, 64], dt.float32, tag="pt")
        nc.tensor.transpose(ptr[:, 0:CR], r1_sb[:, bsl], ident[0:CR, 0:CR])
        nc.tensor.transpose(ptr[:, CR:2 * CR], r2_sb[:, bsl], ident[0:CR, 0:CR])
        nc.vector.tensor_copy(rts[:, b * 2 * CR:(b + 1) * 2 * CR], ptr[:, 0:2 * CR])
    rts3 = rts.rearrange("p (g c) -> p g c", g=2 * NB)
    rmax = st.tile([128, 2 * NB], dt.float32)
    nc.vector.reduce_max(rmax, rts3, axis=AX.X)
    xs_all = st.tile([128, NB * 2 * CR], dt.float32)
    xs3 = xs_all.rearrange("p (g c) -> p g c", g=2 * NB)
    nc.vector.tensor_tensor(
        xs3, rts3, rmax[:, :, None].to_broadcast([128, 2 * NB, CR]), Alu.subtract)
    ex_all = st.tile([128, NB * 2 * CR], dt.float32)
    nc.scalar.activation(ex_all, xs_all, Act.Exp)
    ssum = st.tile([128, 2 * NB], dt.float32)
    nc.vector.reduce_sum(ssum, ex_all.rearrange("p (g c) -> p g c", g=2 * NB),
                         axis=AX.X)
    lse = st.tile([128, 2 * NB], dt.float32)
    nc.scalar.activation(lse, ssum, Act.Ln)
    nc.vector.tensor_tensor(
        xs3, xs3, lse[:, :, None].to_broadcast([128, 2 * NB, CR]), Alu.subtract)
    # xs_all now holds log-probs for both refine heads

    # ---------------- G1 readback + cross-core combine ----------------
    g_sb = st.tile([n_cores, GW], dt.float32)
    nc.sync.dma_start(g_sb, g1_out.rearrange("(r w) -> r w", r=n_cores))
    vmx = st.tile([n_cores, 81], dt.float32)
    nc.gpsimd.partition_all_reduce(
        vmx, g_sb[:, 0:81], channels=n_cores, reduce_op=bass_isa.ReduceOp.max
    )
    selc = st.tile([n_cores, 81], dt.float32)
    nc.vector.tensor_tensor(selc, g_sb[:, 0:81], vmx, Alu.is_equal)
    sel1m = st.tile([n_cores, C], dt.float32)
    nc.vector.tensor_copy(
        sel1m[:, :, None],
        selc[:, 0:60].rearrange("r (c t) -> r c t", t=3)[:, :, 0:1])
    masked = st.tile([n_cores, 160], dt.float32)
    mview = masked.rearrange("p (co s c) -> p co s c", co=4, s=2)
    gview = g_sb[:, 81:241].rearrange("p (co s c) -> p co s c", co=4, s=2)
    nc.vector.tensor_tensor(
        mview[:, :, 0, :], gview[:, :, 0, :],
        sel1m[:, None, :].to_broadcast([n_cores, 4, C]), Alu.mult,
    )
    nc.vector.tensor_tensor(
        mview[:, :, 1, :], gview[:, :, 1, :],
        selc[:, None, 61:81].to_broadcast([n_cores, 4, C]), Alu.mult,
    )
    ps_qr = pss1.tile([128, NS], dt.float32, tag="mm")
    nc.tensor.matmul(ps_qr[0:1, 0:160], ones_col[0:n_cores, :], masked,
                     start=True, stop=True, skip_group_check=True)
    nc.tensor.matmul(ps_qr[0:1, 160:220], ones_col[0:n_cores, :], g_sb[:, 0:60],
                     start=True, stop=True, skip_group_check=True)
    qzs = st.tile([1, 220], dt.float32)
    nc.vector.tensor_copy(qzs, ps_qr[0:1, 0:220])

    # broadcast [boxes(160) | mask(40)] to all 128 partitions
    ps_q = pss1.tile([128, NS], dt.float32, tag="mm")
    nc.tensor.matmul(ps_q[:, 0:160], ones_row[0:1, :], qzs[:, 0:160],
                     start=True, stop=True, skip_group_check=True)
    nc.tensor.matmul(ps_q[:, 160:200], ones_row[0:1, :], mask_row,
                     start=True, stop=True, skip_group_check=True)
    # materialize NB-tiled copy so later ops use plain strided in0 views
    Q4 = st.tile([128, NB, 200], dt.float32)
    nc.vector.tensor_copy(
        Q4, ps_q[:, None, 0:200].to_broadcast([128, NB, 200]))

    # query areas + roi areas [128, NB, 40]
    aqt = stp.tile([128, NB, W2], dt.float32, tag="aqt")
    nc.vector.scalar_tensor_tensor(
        aqt, Q4[:, :, 80:120], 1.0, Q4[:, :, 0:40], Alu.add, Alu.subtract)
    ab40 = st.tile([128, NB, W2], dt.float32)
    nc.vector.scalar_tensor_tensor(
        ab40, Q4[:, :, 120:160], 1.0, Q4[:, :, 40:80], Alu.add, Alu.subtract)
    nc.vector.tensor_mul(ab40, ab40, aqt)
    nc.vector.tensor_tensor(
        ab40, ab40, ab_all[:, :, None].to_broadcast([128, NB, W2]), Alu.add)

    # ---------------- batched paired IoU / assignment / loss ----------------
    xi1 = stp.tile([128, NB, W2], dt.float32, tag="xi1")
    nc.vector.tensor_tensor(
        xi1, Q4[:, :, 0:40],
        boxes_nat[:, :, 0:1].to_broadcast([128, NB, W2]), Alu.max)
    yi1 = stp.tile([128, NB, W2], dt.float32, tag="yi1")
    nc.vector.tensor_tensor(
        yi1, Q4[:, :, 40:80],
        boxes_nat[:, :, 1:2].to_broadcast([128, NB, W2]), Alu.max)
    xi2 = stp.tile([128, NB, W2], dt.float32, tag="xi2")
    nc.vector.tensor_tensor(
        xi2, Q4[:, :, 80:120],
        boxes_nat[:, :, 2:3].to_broadcast([128, NB, W2]), Alu.min)
    yi2 = stp.tile([128, NB, W2], dt.float32, tag="yi2")
    nc.vector.tensor_tensor(
        yi2, Q4[:, :, 120:160],
        boxes_nat[:, :, 3:4].to_broadcast([128, NB, W2]), Alu.min)
    nc.vector.scalar_tensor_tensor(xi2, xi2, 1.0, xi1, Alu.add, Alu.subtract)
    nc.vector.tensor_scalar_max(xi2, xi2, 0.0)   # iw
    nc.vector.scalar_tensor_tensor(yi2, yi2, 1.0, yi1, Alu.add, Alu.subtract)
    nc.vector.tensor_scalar_max(yi2, yi2, 0.0)   # ih
    inter = stp.tile([128, NB, W2], dt.float32, tag="inter")
    nc.vector.tensor_mul(inter, xi2, yi2)
    un = stp.tile([128, NB, W2], dt.float32, tag="un")
    nc.vector.tensor_sub(un, ab40, inter)
    unf = un.rearrange("p b w -> p (b w)")
    nc.vector.reciprocal(unf, unf)
    ov = stp.tile([128, NB, W2], dt.float32, tag="ov")
    nc.vector.tensor_mul(ov, inter, un)
    # mask image-level negatives to exactly -1: ov = (ov+1)*mask - 1
    nc.vector.scalar_tensor_tensor(
        ov, ov, 1.0, Q4[:, :, 160:200], Alu.add, Alu.mult)
    ovf = ov.rearrange("p b w -> p (b w)")
    nc.vector.tensor_scalar_add(ovf, ovf, -1.0)

    ov4 = ov.rearrange("p b (s c) -> p (b s) c", s=2)
    mo = stp.tile([128, 2 * NB], dt.float32, tag="mo")
    nc.vector.reduce_max(mo, ov4, axis=AX.X)
    meq = stp.tile([128, 2 * NB, C], dt.float32, tag="meq")
    nc.vector.tensor_tensor(
        meq, ov4, mo[:, :, None].to_broadcast([128, 2 * NB, C]), Alu.is_equal)
    nc.vector.tensor_tensor(
        meq, meq, iota_m1k[:, None, :].to_broadcast([128, 2 * NB, C]), Alu.mult)
    gt = stp.tile([128, 2 * NB], dt.float32, tag="gt")
    nc.vector.tensor_reduce(gt, meq, axis=AX.X, op=Alu.min)

    fg = stp.tile([128, 2 * NB], dt.float32, tag="fg")
    nc.vector.tensor_scalar(fg, mo, 0.5, None, Alu.is_gt)
    bgt = stp.tile([128, 2 * NB], dt.float32, tag="bgt")
    nc.vector.tensor_scalar(bgt, mo, 0.5, None, Alu.is_lt)
    bg = stp.tile([128, 2 * NB], dt.float32, tag="bg")
    nc.vector.scalar_tensor_tensor(bg, mo, 0.1, bgt, Alu.is_ge, Alu.mult)
    keep = stp.tile([128, 2 * NB], dt.float32, tag="keep")
    nc.vector.tensor_add(keep, fg, bg)
    col = stp.tile([128, 2 * NB], dt.float32, tag="col")
    nc.vector.scalar_tensor_tensor(col, gt, 1001.0, fg, Alu.add, Alu.mult)

    oh = stp.tile([128, 2 * NB, CR], dt.float32, tag="oh")
    nc.vector.tensor_tensor(
        oh, iota8.rearrange("p (g c) -> p g c", g=2 * NB),
        col[:, :, None].to_broadcast([128, 2 * NB, CR]), Alu.is_equal)
    nc.vector.tensor_tensor(oh, oh, xs3, Alu.mult)
    lpsel = stp.tile([128, 2 * NB], dt.float32, tag="lpsel")
    nc.vector.reduce_sum(lpsel, oh, axis=AX.X)

    wk = st.tile([128, 16], dt.float32)
    wl3 = wk[:, 0:8].rearrange("p (b s) -> p b s", s=2)
    nc.vector.tensor_tensor(
        wl3, keep.rearrange("p (b s) -> p b s", s=2),
        isw_col[:, :, None].to_broadcast([128, NB, 2]), Alu.mult)
    nc.vector.tensor_tensor(wk[:, 0:8], wk[:, 0:8], lpsel, Alu.mult)
    nc.vector.tensor_copy(wk[:, 8:16], keep)
    ps_l = pss2.tile([128, 64], dt.float32, tag="acc")
    nc.tensor.matmul(ps_l[0:16, 0:1], wk, ones_col, start=True, stop=True)
    l16 = st.tile([16, 1], dt.float32)
    nc.vector.tensor_copy(l16, ps_l[0:16, 0:1])

    # ---------------- R3: AllGather of loss partials ----------------
    # (an 8-core AllGather is a 2-event mesh vs AllReduce's 5 events; the
    # cross-core sum happens locally with one ones-matmul)
    cc3_in = dp.tile([16], dt.float32)
    cc3_out = dp.tile([n_cores * 16], dt.float32)
    nc.sync.dma_start(cc3_in, l16[:, 0])
    nc.gpsimd.collective_compute(
        "AllGather", Alu.bypass, replica_groups=group,
        ins=[cc3_in.opt()], outs=[cc3_out.opt()],
    )

    # ---- hinge loss from qzs (fills R3 latency) ----
    zsv = qzs[:, 160:220].rearrange("a (c t) -> a c t", t=3)
    zrow = st.tile([1, C], dt.float32)
    nc.vector.tensor_copy(zrow[:, :, None], zsv[:, :, 1:2])
    s1row = st.tile([1, C], dt.float32)
    nc.vector.tensor_copy(s1row[:, :, None], zsv[:, :, 2:3])
    zinv = st.tile([1, C], dt.float32)
    nc.vector.reciprocal(zinv, zrow)
    dcs = st.tile([1, C], dt.float32)
    nc.vector.tensor_mul(dcs, s1row, zinv)
    hv = st.tile([1, C], dt.float32)
    nc.vector.tensor_mul(hv, labrow_f, dcs)
    nc.scalar.activation(hv, hv, Act.Relu, bias=1.0, scale=-1.0)  # relu(1-lab*dcs)
    h = st.tile([1, 1], dt.float32)
    nc.vector.reduce_sum(h, hv, axis=AX.X)
    nc.scalar.mul(h, h, 1.0 / C)

    # ---------------- R3 readback + final scalar ----------------
    g2_sb = st.tile([n_cores, 16], dt.float32)
    nc.sync.dma_start(g2_sb, cc3_out.rearrange("(r w) -> r w", r=n_cores))
    ps_f = pss1.tile([128, NS], dt.float32, tag="mm")
    nc.tensor.matmul(ps_f[0:1, 0:16], ones_col[0:n_cores, :], g2_sb,
                     start=True, stop=True)
    l4 = st.tile([1, 16], dt.float32)
    nc.vector.tensor_copy(l4, ps_f[0:1, 0:16])
    nums = st.tile([1, 2], dt.float32)
    nc.vector.reduce_sum(
        nums, l4[:, 0:8].rearrange("a (b s) -> a s b", s=2), axis=AX.X)
    dens = st.tile([1, 2], dt.float32)
    nc.vector.reduce_sum(
        dens, l4[:, 8:16].rearrange("a (b s) -> a s b", s=2), axis=AX.X)
    dinv = st.tile([1, 2], dt.float32)
    nc.vector.reciprocal(dinv, dens)
    rl = st.tile([1, 2], dt.float32)
    nc.vector.tensor_mul(rl, nums, dinv)
    rsum = st.tile([1, 1], dt.float32)
    nc.vector.reduce_sum(rsum, rl, axis=AX.X)
    tot = st.tile([1, 1], dt.float32)
    nc.scalar.mul(tot, rsum, -0.1)
    nc.vector.tensor_add(tot, tot, h)
    nc.sync.dma_start(loss, tot)

    for pool in (dp, pss2, pss1, pst, natp, stp, st, const):
        pool.release()


def build_program(NS=512, F=4096, n_cores=8):
    nc = bacc.Bacc(
        "TRN2", target_bir_lowering=False, debug=False, num_devices=n_cores
    )
    KT = F // 128
    NB = NS // 128
    bf = dt.bfloat16
    f32 = dt.float32
    roi = nc.dram_tensor("roi", [128, KT, NS], bf, kind="ExternalInput").ap()
    frm = nc.dram_tensor("frm", [128, KT, NS], bf, kind="ExternalInput").ap()
    ctxm = nc.dram_tensor("ctxm", [128, KT, NS], bf, kind="ExternalInput").ap()
    w_all = nc.dram_tensor("w_all", [128, KT, CA], bf, kind="ExternalInput").ap()
    wdc = nc.dram_tensor("wdc", [128, KT, 2, C], bf, kind="ExternalInput").ap()
    b_all = nc.dram_tensor("b_all", [CA], f32, kind="ExternalInput").ap()
    b_det = nc.dram_tensor("b_det", [C], f32, kind="ExternalInput").ap()
    boxes = nc.dram_tensor("boxes", [128, NB, 4], f32, kind="ExternalInput").ap()
    isw = nc.dram_tensor("isw", [NS], f32, kind="ExternalInput").ap()
    iswc = nc.dram_tensor("iswc", [128, NB], f32, kind="ExternalInput").ap()
    lab = nc.dram_tensor("lab", [1, C], dt.int32, kind="ExternalInput").ap()
    loss = nc.dram_tensor("loss", [1, 1], f32, kind="ExternalOutput").ap()
    aps = (roi, frm, ctxm, w_all, wdc, b_all, b_det, boxes, isw, iswc, lab,
           loss)
    with tile.TileContext(nc) as tc:
        _emit(nc, tc, aps, NS, F, n_cores)
    nc.compile()
    return nc


def make_in_maps(inputs, NS, n_cores):
    f32 = np.float32
    bf = ml_dtypes.bfloat16
    F = inputs["fc7_roi"].shape[1]
    KT = F // 128
    NB = NS // 128

    def _wpack(w):
        # [F, cols] -> [128, KT, cols] contiguous
        return np.ascontiguousarray(
            np.asarray(w, f32).reshape(KT, 128, -1).transpose(1, 0, 2)
        ).astype(bf)

    w_all = _wpack(np.concatenate(
        [np.asarray(inputs["W_cls"]), np.asarray(inputs["W_r1"]),
         np.asarray(inputs["W_r2"])], axis=1))
    wd = np.asarray(inputs["W_det"], f32)
    wdc = np.ascontiguousarray(
        np.stack([wd.reshape(KT, 128, C), -wd.reshape(KT, 128, C)], axis=2)
        .transpose(1, 0, 2, 3)).astype(bf)
    b_all = np.ascontiguousarray(
        np.concatenate([np.asarray(inputs["b_cls"]), np.asarray(inputs["b_r1"]),
                        np.asarray(inputs["b_r2"])]), f32)
    b_det = np.ascontiguousarray(np.asarray(inputs["b_det"]), f32)
    boxes = np.asarray(inputs["ss_boxes"], f32)[:, 1:5]
    isw = np.ascontiguousarray(np.asarray(inputs["IS_weight"])[:, 0], f32)
    lab = np.ascontiguousarray(np.asarray(inputs["image_level_label"]), np.int32)
    roi = np.asarray(inputs["fc7_roi"], f32).T.astype(bf)
    frm = np.asarray(inputs["fc7_frame"], f32).T.astype(bf)
    ctxm = np.asarray(inputs["fc7_context"], f32).T.astype(bf)

    def _pack(a, sl):
        # [F, NS] slice -> [128, KT, NS]: 8KB-contiguous per-partition runs
        return np.ascontiguousarray(
            a[:, sl].reshape(KT, 128, -1).transpose(1, 0, 2))

    in_maps = []
    for c in range(n_cores):
        sl = slice(c * NS, (c + 1) * NS)
        bsl = boxes[sl]
        isl = isw[sl]
        in_maps.append({
            "roi": _pack(roi, sl),
            "frm": _pack(frm, sl),
            "ctxm": _pack(ctxm, sl),
            "w_all": w_all, "wdc": wdc, "b_all": b_all, "b_det": b_det,
            "boxes": np.ascontiguousarray(
                bsl.reshape(NB, 128, 4).transpose(1, 0, 2)),
            "isw": isl,
            "iswc": np.ascontiguousarray(isl.reshape(NB, 128).T),
            "lab": lab,
        })
    return in_maps


_PROG_CACHE = {}


def _get_prog(NS, F, n_cores):
    key = (NS, F, n_cores)
    if key not in _PROG_CACHE:
        _PROG_CACHE[key] = build_program(NS, F, n_cores)
    return _PROG_CACHE[key]


def kernel(**inputs):
    n_cores = 8
    N, F = inputs["fc7_roi"].shape
    NS = N // n_cores
    prog = _get_prog(NS, F, n_cores)
    in_maps = make_in_maps(inputs, NS, n_cores)
    res = run_bass_kernel_spmd(prog, in_maps, list(range(n_cores))).results
    return np.float32(res[0]["loss"].reshape(()))


# revision 29
# speedup vs baseline: 1.6675x; 1.0286x over previous
# Trainium2 Bass kernel for nn_Network_515396076038 (nms_detection / OICR-style loss).
#
# Strategy (8 NeuronCores, data-parallel over the N=4096 proposals):
#   - Each core owns NS = N/8 = 512 rois and streams its shard of fc7_roi /
#     fc7_frame / fc7_context from HBM in bf16 (12 MB per core -> memory
#     roofline ~34us), computing the GEMM heads in class-major layout [C, NS]
#     (scores^T = W^T @ X^T accumulated over F).  det = W@frm - W@ctx is
#     computed on the PE by accumulating ctx with -W (host-negated weights),
#     so the vector engine does no work during the stream.
#   - Per-class argmax over rois is computed in LOG domain:
#     log p = cls + det + ln(isw) - ln(sum_c exp(cls)) -- the global det
#     softmax normalizer is a per-class constant and cannot change the
#     argmax, and the log form avoids expensive vector reciprocals.
#   - One AllGather ships each core's per-class maxima + candidate boxes
#     (sel-mask matmul gather) + softmax partial sums; refine-head
#     log-softmax prep fills the collective latency.  A final tiny
#     AllReduce(add) sums 16 loss partials (4 roi-blocks x 2 supervisions x
#     {num, den}); the per-block sums ride the collective for free.
#   - IoU / fg-bg assignment / one-hot log-prob gather run roi-major,
#     batched across all 4 roi-blocks and both supervisions in single
#     vector ops on [128, 160] tiles via stride-0 broadcast views.
import sys

for _p in ("/opt/trn_rl_repo",):
    if _p not in sys.path:
        sys.path.append(_p)

import numpy as np
import ml_dtypes

import concourse.bass as bass
import concourse.bass_isa as bass_isa
import concourse.mybir as mybir
import concourse.tile as tile
from concourse import bacc
from concourse.bass_utils import run_bass_kernel_spmd
from concourse.masks import make_identity

dt = mybir.dt
Alu = mybir.AluOpType
Act = mybir.ActivationFunctionType
AX = mybir.AxisListType

C = 20      # foreground classes
CR = C + 1  # refine head classes (background + C)
CA = C + 2 * CR  # stacked roi-head outputs: cls | r1 | r2 = 62
W2 = 2 * C


def _emit(nc, tc, aps, NS, F, n_cores):
    NB = NS // 128
    KT = F // 128
    group = [list(range(n_cores))]
    GW = 241  # AllGather row: vm1[20] vm2[21] boxes[160] z[20] s1[20]

    (roi, frm, ctxm, w_all, wdc, b_all, b_det, boxes, isw, iswc, lab,
     loss) = aps

    const = tc.alloc_tile_pool(name="const", bufs=1)
    st = tc.alloc_tile_pool(name="st", bufs=1)
    stp = tc.alloc_tile_pool(name="stp", bufs=2)
    natp = tc.alloc_tile_pool(name="natp", bufs=2)
    pst = tc.alloc_tile_pool(name="pst", bufs=2, space="PSUM")
    pss1 = tc.alloc_tile_pool(name="pss1", bufs=2, space="PSUM")
    pss2 = tc.alloc_tile_pool(name="pss2", bufs=1, space="PSUM")
    dp = tc.alloc_tile_pool(name="dp", bufs=1, space="DRAM")
    # psc is created last: it is the first pool released (LIFO pool stack)
    psc = tc.alloc_tile_pool(name="psc", bufs=1, space="PSUM")

    # ---- big stream DMAs first: weights + first super-tile ----
    KS = min(8, KT)
    SK = KT // KS
    w_all_sb = const.tile([128, KT, CA], dt.bfloat16)
    nc.sync.dma_start(w_all_sb, w_all)
    wdc_sb = const.tile([128, KT, 2, C], dt.bfloat16)
    nc.sync.dma_start(wdc_sb, wdc)
    nat0 = []
    for nm, srcp in (("roi", roi), ("frm", frm), ("ctx", ctxm)):
        t = natp.tile([128, KS, NS], dt.bfloat16, tag=nm)
        nc.sync.dma_start(t, srcp[:, bass.ts(0, KS), :])
        nat0.append(t)

    # ---------------- constants / setup (overlaps the GEMM stream) --------
    ident = const.tile([128, 128], dt.float32)
    make_identity(nc, ident)
    ones_col = const.tile([128, 1], dt.float32)
    nc.vector.memset(ones_col, 1.0)
    ones_row = const.tile([1, 128], dt.float32)
    nc.vector.memset(ones_row, 1.0)
    iota_i = const.tile([128, CR], dt.int32)
    nc.gpsimd.iota(iota_i, pattern=[[1, CR]], base=0, channel_multiplier=0)
    iota_f = const.tile([128, CR], dt.float32)
    nc.vector.tensor_copy(iota_f, iota_i)
    iota_m1k = const.tile([128, C], dt.float32)
    nc.vector.tensor_scalar_add(iota_m1k, iota_f[:, :C], -1000.0)
    iota8 = const.tile([128, 2 * NB * CR], dt.float32)
    nc.vector.tensor_copy(
        iota8.rearrange("p (b c) -> p b c", b=2 * NB),
        iota_f[:, None, :].to_broadcast([128, 2 * NB, CR]),
    )
    # pre-warm the exp activation table while the stream runs
    pwz = const.tile([1, 1], dt.float32)
    nc.vector.memset(pwz, 0.0)
    pw = const.tile([1, 1], dt.float32)
    nc.scalar.activation(pw, pwz, Act.Exp)

    b_all_sb = const.tile([CA, 1], dt.float32)
    nc.sync.dma_start(b_all_sb, b_all[:, None])
    b_det_sb = const.tile([C, 1], dt.float32)
    nc.sync.dma_start(b_det_sb, b_det[:, None])

    labrow_i = st.tile([1, C], dt.int32)
    nc.sync.dma_start(labrow_i, lab)
    labrow_f = st.tile([1, C], dt.float32)
    nc.vector.tensor_copy(labrow_f, labrow_i)
    mask_row = st.tile([1, W2], dt.float32)
    nc.vector.tensor_scalar(mask_row[:, 0:C], labrow_f, 1.0, None, Alu.is_equal)
    nc.vector.tensor_copy(mask_row[:, C:W2], mask_row[:, 0:C])

    isw_row = st.tile([1, NS], dt.float32)
    nc.sync.dma_start(isw_row, isw[None, :])
    isw_col = st.tile([128, NB], dt.float32)
    nc.sync.dma_start(isw_col, iswc)
    boxes_nat = st.tile([128, NB, 4], dt.float32)
    nc.sync.dma_start(boxes_nat, boxes)

    # per-roi box areas [128, NB]
    ab_all = st.tile([128, NB], dt.float32)
    abt = stp.tile([128, NB], dt.float32, tag="abt")
    nc.vector.scalar_tensor_tensor(
        abt, boxes_nat[:, :, 2], 1.0, boxes_nat[:, :, 0], Alu.add, Alu.subtract)
    nc.vector.scalar_tensor_tensor(
        ab_all, boxes_nat[:, :, 3], 1.0, boxes_nat[:, :, 1], Alu.add, Alu.subtract)
    nc.vector.tensor_mul(ab_all, ab_all, abt)

    # ---------------- main GEMM phase (bf16, DMA-bound) ----------------
    # scoresA rows: [0:C] cls, [C:C+CR] r1, [C+CR:CA] r2 ; scoresB rows [0:C] det
    scoresA = psc.tile([128, NS], dt.float32)
    scoresB = psc.tile([128, NS], dt.float32)

    for sk in range(SK):
        if sk == 0:
            t_roi, t_frm, t_ctx = nat0
        else:
            ksl = bass.ts(sk, KS)
            t_roi = natp.tile([128, KS, NS], dt.bfloat16, tag="roi")
            nc.sync.dma_start(t_roi, roi[:, ksl, :])
            t_frm = natp.tile([128, KS, NS], dt.bfloat16, tag="frm")
            nc.sync.dma_start(t_frm, frm[:, ksl, :])
            t_ctx = natp.tile([128, KS, NS], dt.bfloat16, tag="ctx")
            nc.sync.dma_start(t_ctx, ctxm[:, ksl, :])
        for j in range(KS):
            k = sk * KS + j
            nc.tensor.matmul(
                scoresA[0:CA, :], w_all_sb[:, k, :], t_roi[:, j, :],
                start=(k == 0), stop=(k == KT - 1),
            )
        for j in range(KS):
            k = sk * KS + j
            nc.tensor.matmul(
                scoresB[0:C, :], wdc_sb[:, k, 0, :], t_frm[:, j, :],
                start=(k == 0), stop=False,
            )
        for j in range(KS):
            k = sk * KS + j
            nc.tensor.matmul(
                scoresB[0:C, :], wdc_sb[:, k, 1, :], t_ctx[:, j, :],
                start=False, stop=(k == KT - 1),
            )

    # ---------------- class-major stats (log-domain argmax) ----------------
    stk = st.tile([CA, NS], dt.float32)
    nc.scalar.activation(stk, scoresA[0:CA, :], Act.Identity, bias=b_all_sb)
    det_sb = st.tile([C, NS], dt.float32)
    nc.scalar.activation(det_sb, scoresB[0:C, :], Act.Identity, bias=b_det_sb)
    psc.release()
    # de-stack r1/r2 to partition base 0 (SBUF->SBUF DMA moves partitions)
    r1_sb = st.tile([CR, NS], dt.float32)
    nc.sync.dma_start(r1_sb, stk[C:C + CR, :])
    r2_sb = st.tile([CR, NS], dt.float32)
    nc.sync.dma_start(r2_sb, stk[C + CR:CA, :])

    # vmzs cols: 0 = per-class max (filled later), 1 = z partial, 2 = s1
    vmzs = st.tile([C, 3], dt.float32)
    exp_det = st.tile([C, NS], dt.float32)
    nc.scalar.activation(exp_det, det_sb, Act.Exp, accum_out=vmzs[:, 1:2])
    prod_cd = st.tile([C, NS], dt.float32)
    nc.vector.tensor_mul(prod_cd, stk[0:C, :], exp_det)
    nc.vector.reduce_sum(vmzs[:, 2:3], prod_cd, axis=AX.X)

    exp_cls = st.tile([C, NS], dt.float32)
    nc.scalar.activation(exp_cls, stk[0:C, :], Act.Exp)
    exp_r1 = st.tile([CR, NS], dt.float32)
    nc.scalar.activation(exp_r1, r1_sb, Act.Exp)

    ps_s1 = pss1.tile([128, NS], dt.float32, tag="mm")
    nc.tensor.matmul(ps_s1[0:1, 0:NS], ones_col[0:C, :], exp_cls,
                     start=True, stop=True)
    ps_s2 = pss1.tile([128, NS], dt.float32, tag="mm")
    nc.tensor.matmul(ps_s2[0:1, 0:NS], ones_col[0:CR, :], exp_r1,
                     start=True, stop=True)
    # trow = ln(isw) - ln(normalizer), per head.  Both normalizers go
    # through ONE Ln activation so the scheduler cannot interleave exp/ln
    # table loads (the combined Ln depends on both exp matmuls).
    ln_isw = st.tile([1, NS], dt.float32)
    nc.scalar.activation(ln_isw, isw_row, Act.Ln)
    lnin = st.tile([1, 2 * NS], dt.float32)
    nc.vector.tensor_copy(lnin[:, 0:NS], ps_s1[0:1, 0:NS])
    nc.vector.tensor_copy(lnin[:, NS:2 * NS], ps_s2[0:1, 0:NS])
    lnout = st.tile([1, 2 * NS], dt.float32)
    nc.scalar.activation(lnout, lnin, Act.Ln)
    trow1 = st.tile([1, NS], dt.float32)
    nc.vector.tensor_sub(trow1, ln_isw, lnout[:, 0:NS])
    trow2 = st.tile([1, NS], dt.float32)
    nc.vector.tensor_sub(trow2, ln_isw, lnout[:, NS:2 * NS])

    ps_b1 = pss1.tile([128, NS], dt.float32, tag="mm")
    nc.tensor.matmul(ps_b1[0:C, 0:NS], ones_row[:, 0:C], trow1,
                     start=True, stop=True)
    ps_b2 = pss1.tile([128, NS], dt.float32, tag="mm")
    nc.tensor.matmul(ps_b2[0:CR, 0:NS], ones_row[:, 0:CR], trow2,
                     start=True, stop=True)

    # lp1 = cls + det + trow1 ; lq2 = r1 + trow2  (argmax-equivalent logs)
    lp1 = st.tile([C, NS], dt.float32)
    nc.vector.tensor_add(lp1, stk[0:C, :], det_sb)
    nc.vector.tensor_add(lp1, lp1, ps_b1[0:C, 0:NS])
    lq2 = st.tile([CR, NS], dt.float32)
    nc.vector.tensor_add(lq2, r1_sb, ps_b2[0:CR, 0:NS])

    nc.vector.reduce_max(vmzs[:, 0:1], lp1, axis=AX.X)
    vm1 = vmzs[:, 0:1]
    vm2 = st.tile([CR, 1], dt.float32)
    nc.vector.reduce_max(vm2, lq2, axis=AX.X)

    # local per-class argmax boxes via sel-mask matmuls (exact one-hot gather)
    sel1 = st.tile([C, NS], dt.float32)
    nc.vector.tensor_scalar(sel1, lp1, vmzs[:, 0:1], None, Alu.is_equal)
    sel2 = st.tile([CR, NS], dt.float32)
    nc.vector.tensor_scalar(sel2, lq2, vm2, None, Alu.is_equal)
    psq = pss2.tile([128, 64], dt.float32, tag="acc")
    for b in range(NB):
        bsl = bass.ts(b, 128)
        ptx = pst.tile([128, 64], dt.float32, tag="pt")
        nc.tensor.transpose(ptx[:, 0:C], sel1[:, bsl], ident[0:C, 0:C])
        nc.tensor.transpose(ptx[:, 32:32 + CR], sel2[:, bsl], ident[0:CR, 0:CR])
        sT = stp.tile([128, 64], dt.float32, tag="sT")
        nc.vector.tensor_copy(sT[:, 0:C], ptx[:, 0:C])
        nc.vector.tensor_copy(sT[:, 32:32 + CR], ptx[:, 32:32 + CR])
        nc.tensor.matmul(
            psq[0:4, 0:C], boxes_nat[:, b, :], sT[:, 0:C],
            start=(b == 0), stop=(b == NB - 1), skip_group_check=True,
        )
        nc.tensor.matmul(
            psq[0:4, C:W2], boxes_nat[:, b, :], sT[:, 33:33 + C],
            start=(b == 0), stop=(b == NB - 1), skip_group_check=True,
        )
    bc_sb = st.tile([4, W2], dt.float32)
    nc.vector.tensor_copy(bc_sb, psq[0:4, 0:W2])

    # ---------------- G1: AllGather of all cross-core state ----------------
    g1_in = dp.tile([GW], dt.float32)
    g1_out = dp.tile([n_cores * GW], dt.float32)
    nc.sync.dma_start(g1_in[0:60], vmzs)
    nc.sync.dma_start(g1_in[60:81], vm2[:, 0])
    nc.sync.dma_start(g1_in[81:241], bc_sb)
    nc.gpsimd.collective_compute(
        "AllGather", Alu.bypass, replica_groups=group,
        ins=[g1_in.opt()], outs=[g1_out.opt()],
    )

    # ---- collective-independent prep, emitted here to fill G1 latency ----
    # refine-head scores to roi-major [128, (b,s), CR], then log-softmax
    rts = st.tile([128, NB * 2 * CR], dt.float32)
    for b in range(NB):
        bsl = bass.ts(b, 128)
        ptr = pst.tile([128, 64], dt.float32, tag="pt")
        nc.tensor.transpose(ptr[:, 0:CR], r1_sb[:, bsl], ident[0:CR, 0:CR])
        nc.tensor.transpose(ptr[:, CR:2 * CR], r2_sb[:, bsl], ident[0:CR, 0:CR])
        nc.vector.tensor_copy(rts[:, b * 2 * CR:(b + 1) * 2 * CR], ptr[:, 0:2 * CR])
    rts3 = rts.rearrange("p (g c) -> p g c", g=2 * NB)
    rmax = st.tile([128, 2 * NB], dt.float32)
    nc.vector.reduce_max(rmax, rts3, axis=AX.X)
    xs_all = st.tile([128, NB * 2 * CR], dt.float32)
    xs3 = xs_all.rearrange("p (g c) -> p g c", g=2 * NB)
    nc.vector.tensor_tensor(
        xs3, rts3, rmax[:, :, None].to_broadcast([128, 2 * NB, CR]), Alu.subtract)
    ex_all = st.tile([128, NB * 2 * CR], dt.float32)
    nc.scalar.activation(ex_all, xs_all, Act.Exp)
    ssum = st.tile([128, 2 * NB], dt.float32)
    nc.vector.reduce_sum(ssum, ex_all.rearrange("p (g c) -> p g c", g=2 * NB),
                         axis=AX.X)
    lse = st.tile([128, 2 * NB], dt.float32)
    nc.scalar.activation(lse, ssum, Act.Ln)
    nc.vector.tensor_tensor(
        xs3, xs3, lse[:, :, None].to_broadcast([128, 2 * NB, CR]), Alu.subtract)
    # xs_all now holds log-probs for both refine heads

    # ---------------- G1 readback + cross-core combine ----------------
    g_sb = st.tile([n_cores, GW], dt.float32)
    nc.sync.dma_start(g_sb, g1_out.rearrange("(r w) -> r w", r=n_cores))
    vmx = st.tile([n_cores, 81], dt.float32)
    nc.gpsimd.partition_all_reduce(
        vmx, g_sb[:, 0:81], channels=n_cores, reduce_op=bass_isa.ReduceOp.max
    )
    selc = st.tile([n_cores, 81], dt.float32)
    nc.vector.tensor_tensor(selc, g_sb[:, 0:81], vmx, Alu.is_equal)
    sel1m = st.tile([n_cores, C], dt.float32)
    nc.vector.tensor_copy(
        sel1m[:, :, None],
        selc[:, 0:60].rearrange("r (c t) -> r c t", t=3)[:, :, 0:1])
    masked = st.tile([n_cores, 160], dt.float32)
    mview = masked.rearrange("p (co s c) -> p co s c", co=4, s=2)
    gview = g_sb[:, 81:241].rearrange("p (co s c) -> p co s c", co=4, s=2)
    nc.vector.tensor_tensor(
        mview[:, :, 0, :], gview[:, :, 0, :],
        sel1m[:, None, :].to_broadcast([n_cores, 4, C]), Alu.mult,
    )
    nc.vector.tensor_tensor(
        mview[:, :, 1, :], gview[:, :, 1, :],
        selc[:, None, 61:81].to_broadcast([n_cores, 4, C]), Alu.mult,
    )
    ps_qr = pss1.tile([128, NS], dt.float32, tag="mm")
    nc.tensor.matmul(ps_qr[0:1, 0:160], ones_col[0:n_cores, :], masked,
                     start=True, stop=True, skip_group_check=True)
    nc.tensor.matmul(ps_qr[0:1, 160:220], ones_col[0:n_cores, :], g_sb[:, 0:60],
                     start=True, stop=True, skip_group_check=True)
    qzs = st.tile([1, 220], dt.float32)
    nc.vector.tensor_copy(qzs, ps_qr[0:1, 0:220])

    # broadcast [boxes(160) | mask(40)] to all 128 partitions
    ps_q = pss1.tile([128, NS], dt.float32, tag="mm")
    nc.tensor.matmul(ps_q[:, 0:160], ones_row[0:1, :], qzs[:, 0:160],
                     start=True, stop=True, skip_group_check=True)
    nc.tensor.matmul(ps_q[:, 160:200], ones_row[0:1, :], mask_row,
                     start=True, stop=True, skip_group_check=True)
    # materialize NB-tiled copy so later ops use plain strided in0 views
    Q4 = st.tile([128, NB, 200], dt.float32)
    nc.vector.tensor_copy(
        Q4, ps_q[:, None, 0:200].to_broadcast([128, NB, 200]))

    # query areas + roi areas [128, NB, 40]
    aqt = stp.tile([128, NB, W2], dt.float32, tag="aqt")
    nc.vector.scalar_tensor_tensor(
        aqt, Q4[:, :, 80:120], 1.0, Q4[:, :, 0:40], Alu.add, Alu.subtract)
    ab40 = st.tile([128, NB, W2], dt.float32)
    nc.vector.scalar_tensor_tensor(
        ab40, Q4[:, :, 120:160], 1.0, Q4[:, :, 40:80], Alu.add, Alu.subtract)
    nc.vector.tensor_mul(ab40, ab40, aqt)
    nc.vector.tensor_tensor(
        ab40, ab40, ab_all[:, :, None].to_broadcast([128, NB, W2]), Alu.add)

    # ---------------- batched paired IoU / assignment / loss ----------------
    xi1 = stp.tile([128, NB, W2], dt.float32, tag="xi1")
    nc.vector.tensor_tensor(
        xi1, Q4[:, :, 0:40],
        boxes_nat[:, :, 0:1].to_broadcast([128, NB, W2]), Alu.max)
    yi1 = stp.tile([128, NB, W2], dt.float32, tag="yi1")
    nc.vector.tensor_tensor(
        yi1, Q4[:, :, 40:80],
        boxes_nat[:, :, 1:2].to_broadcast([128, NB, W2]), Alu.max)
    xi2 = stp.tile([128, NB, W2], dt.float32, tag="xi2")
    nc.vector.tensor_tensor(
        xi2, Q4[:, :, 80:120],
        boxes_nat[:, :, 2:3].to_broadcast([128, NB, W2]), Alu.min)
    yi2 = stp.tile([128, NB, W2], dt.float32, tag="yi2")
    nc.vector.tensor_tensor(
        yi2, Q4[:, :, 120:160],
        boxes_nat[:, :, 3:4].to_broadcast([128, NB, W2]), Alu.min)
    nc.vector.scalar_tensor_tensor(xi2, xi2, 1.0, xi1, Alu.add, Alu.subtract)
    nc.vector.tensor_scalar_max(xi2, xi2, 0.0)   # iw
    nc.vector.scalar_tensor_tensor(yi2, yi2, 1.0, yi1, Alu.add, Alu.subtract)
    nc.vector.tensor_scalar_max(yi2, yi2, 0.0)   # ih
    inter = stp.tile([128, NB, W2], dt.float32, tag="inter")
    nc.vector.tensor_mul(inter, xi2, yi2)
    un = stp.tile([128, NB, W2], dt.float32, tag="un")
    nc.vector.tensor_sub(un, ab40, inter)
    unf = un.rearrange("p b w -> p (b w)")
    nc.vector.reciprocal(unf, unf)
    ov = stp.tile([128, NB, W2], dt.float32, tag="ov")
    nc.vector.tensor_mul(ov, inter, un)
    # mask image-level negatives to exactly -1: ov = (ov+1)*mask - 1
    nc.vector.scalar_tensor_tensor(
        ov, ov, 1.0, Q4[:, :, 160:200], Alu.add, Alu.mult)
    ovf = ov.rearrange("p b w -> p (b w)")
    nc.vector.tensor_scalar_add(ovf, ovf, -1.0)

    ov4 = ov.rearrange("p b (s c) -> p (b s) c", s=2)
    mo = stp.tile([128, 2 * NB], dt.float32, tag="mo")
    nc.vector.reduce_max(mo, ov4, axis=AX.X)
    meq = stp.tile([128, 2 * NB, C], dt.float32, tag="meq")
    nc.vector.tensor_tensor(
        meq, ov4, mo[:, :, None].to_broadcast([128, 2 * NB, C]), Alu.is_equal)
    nc.vector.tensor_tensor(
        meq, meq, iota_m1k[:, None, :].to_broadcast([128, 2 * NB, C]), Alu.mult)
    gt = stp.tile([128, 2 * NB], dt.float32, tag="gt")
    nc.vector.tensor_reduce(gt, meq, axis=AX.X, op=Alu.min)

    fg = stp.tile([128, 2 * NB], dt.float32, tag="fg")
    nc.vector.tensor_scalar(fg, mo, 0.5, None, Alu.is_gt)
    bgt = stp.tile([128, 2 * NB], dt.float32, tag="bgt")
    nc.vector.tensor_scalar(bgt, mo, 0.5, None, Alu.is_lt)
    bg = stp.tile([128, 2 * NB], dt.float32, tag="bg")
    nc.vector.scalar_tensor_tensor(bg, mo, 0.1, bgt, Alu.is_ge, Alu.mult)
    keep = stp.tile([128, 2 * NB], dt.float32, tag="keep")
    nc.vector.tensor_add(keep, fg, bg)
    col = stp.tile([128, 2 * NB], dt.float32, tag="col")
    nc.vector.scalar_tensor_tensor(col, gt, 1001.0, fg, Alu.add, Alu.mult)

    oh = stp.tile([128, 2 * NB, CR], dt.float32, tag="oh")
    nc.vector.tensor_tensor(
        oh, iota8.rearrange("p (g c) -> p g c", g=2 * NB),
        col[:, :, None].to_broadcast([128, 2 * NB, CR]), Alu.is_equal)
    nc.vector.tensor_tensor(oh, oh, xs3, Alu.mult)
    lpsel = stp.tile([128, 2 * NB], dt.float32, tag="lpsel")
    nc.vector.reduce_sum(lpsel, oh, axis=AX.X)

    wk = st.tile([128, 16], dt.float32)
    wl3 = wk[:, 0:8].rearrange("p (b s) -> p b s", s=2)
    nc.vector.tensor_tensor(
        wl3, keep.rearrange("p (b s) -> p b s", s=2),
        isw_col[:, :, None].to_broadcast([128, NB, 2]), Alu.mult)
    nc.vector.tensor_tensor(wk[:, 0:8], wk[:, 0:8], lpsel, Alu.mult)
    nc.vector.tensor_copy(wk[:, 8:16], keep)
    ps_l = pss2.tile([128, 64], dt.float32, tag="acc")
    nc.tensor.matmul(ps_l[0:16, 0:1], wk, ones_col, start=True, stop=True)
    l16 = st.tile([16, 1], dt.float32)
    nc.vector.tensor_copy(l16, ps_l[0:16, 0:1])

    # ---------------- R3: AllGather of loss partials ----------------
    # (an 8-core AllGather is a 2-event mesh vs AllReduce's 5 events; the
    # cross-core sum happens locally with one ones-matmul)
    cc3_in = dp.tile([16], dt.float32)
    cc3_out = dp.tile([n_cores * 16], dt.float32)
    nc.sync.dma_start(cc3_in, l16[:, 0])
    nc.gpsimd.collective_compute(
        "AllGather", Alu.bypass, replica_groups=group,
        ins=[cc3_in.opt()], outs=[cc3_out.opt()],
    )

    # ---- hinge loss from qzs (fills R3 latency) ----
    zsv = qzs[:, 160:220].rearrange("a (c t) -> a c t", t=3)
    zrow = st.tile([1, C], dt.float32)
    nc.vector.tensor_copy(zrow[:, :, None], zsv[:, :, 1:2])
    s1row = st.tile([1, C], dt.float32)
    nc.vector.tensor_copy(s1row[:, :, None], zsv[:, :, 2:3])
    zinv = st.tile([1, C], dt.float32)
    nc.vector.reciprocal(zinv, zrow)
    dcs = st.tile([1, C], dt.float32)
    nc.vector.tensor_mul(dcs, s1row, zinv)
    hv = st.tile([1, C], dt.float32)
    nc.vector.tensor_mul(hv, labrow_f, dcs)
    nc.scalar.activation(hv, hv, Act.Relu, bias=1.0, scale=-1.0)  # relu(1-lab*dcs)
    h = st.tile([1, 1], dt.float32)
    nc.vector.reduce_sum(h, hv, axis=AX.X)
    nc.scalar.mul(h, h, 1.0 / C)

    # ---------------- R3 readback + final scalar ----------------
    g2_sb = st.tile([n_cores, 16], dt.float32)
    nc.sync.dma_start(g2_sb, cc3_out.rearrange("(r w) -> r w", r=n_cores))
    ps_f = pss1.tile([128, NS], dt.float32, tag="mm")
    nc.tensor.matmul(ps_f[0:1, 0:16], ones_col[0:n_cores, :], g2_sb,
                     start=True, stop=True)
    l4 = st.tile([1, 16], dt.float32)
    nc.vector.tensor_copy(l4, ps_f[0:1, 0:16])
    nums = st.tile([1, 2], dt.float32)
    nc.vector.reduce_sum(
        nums, l4[:, 0:8].rearrange("a (b s) -> a s b", s=2), axis=AX.X)
    dens = st.tile([1, 2], dt.float32)
    nc.vector.reduce_sum(
        dens, l4[:, 8:16].rearrange("a (b s) -> a s b", s=2), axis=AX.X)
    dinv = st.tile([1, 2], dt.float32)
    nc.vector.reciprocal(dinv, dens)
    rl = st.tile([1, 2], dt.float32)
    nc.vector.tensor_mul(rl, nums, dinv)
    rsum = st.tile([1, 1], dt.float32)
    nc.vector.reduce_sum(rsum, rl, axis=AX.X)
    tot = st.tile([1, 1], dt.float32)
    nc.scalar.mul(tot, rsum, -0.1)
    nc.vector.tensor_add(tot, tot, h)
    nc.sync.dma_start(loss, tot)

    for pool in (dp, pss2, pss1, pst, natp, stp, st, const):
        pool.release()


def build_program(NS=512, F=4096, n_cores=8):
    nc = bacc.Bacc(
        "TRN2", target_bir_lowering=False, debug=False, num_devices=n_cores
    )
    KT = F // 128
    NB = NS // 128
    bf = dt.bfloat16
    f32 = dt.float32
    roi = nc.dram_tensor("roi", [128, KT, NS], bf, kind="ExternalInput").ap()
    frm = nc.dram_tensor("frm", [128, KT, NS], bf, kind="ExternalInput").ap()
    ctxm = nc.dram_tensor("ctxm", [128, KT, NS], bf, kind="ExternalInput").ap()
    w_all = nc.dram_tensor("w_all", [128, KT, CA], bf, kind="ExternalInput").ap()
    wdc = nc.dram_tensor("wdc", [128, KT, 2, C], bf, kind="ExternalInput").ap()
    b_all = nc.dram_tensor("b_all", [CA], f32, kind="ExternalInput").ap()
    b_det = nc.dram_tensor("b_det", [C], f32, kind="ExternalInput").ap()
    boxes = nc.dram_tensor("boxes", [128, NB, 4], f32, kind="ExternalInput").ap()
    isw = nc.dram_tensor("isw", [NS], f32, kind="ExternalInput").ap()
    iswc = nc.dram_tensor("iswc", [128, NB], f32, kind="ExternalInput").ap()
    lab = nc.dram_tensor("lab", [1, C], dt.int32, kind="ExternalInput").ap()
    loss = nc.dram_tensor("loss", [1, 1], f32, kind="ExternalOutput").ap()
    aps = (roi, frm, ctxm, w_all, wdc, b_all, b_det, boxes, isw, iswc, lab,
           loss)
    with tile.TileContext(nc) as tc:
        _emit(nc, tc, aps, NS, F, n_cores)
    nc.compile()
    return nc


def make_in_maps(inputs, NS, n_cores):
    f32 = np.float32
    bf = ml_dtypes.bfloat16
    F = inputs["fc7_roi"].shape[1]
    KT = F // 128
    NB = NS // 128

    def _wpack(w):
        # [F, cols] -> [128, KT, cols] contiguous
        return np.ascontiguousarray(
            np.asarray(w, f32).reshape(KT, 128, -1).transpose(1, 0, 2)
        ).astype(bf)

    w_all = _wpack(np.concatenate(
        [np.asarray(inputs["W_cls"]), np.asarray(inputs["W_r1"]),
         np.asarray(inputs["W_r2"])], axis=1))
    wd = np.asarray(inputs["W_det"], f32)
    wdc = np.ascontiguousarray(
        np.stack([wd.reshape(KT, 128, C), -wd.reshape(KT, 128, C)], axis=2)
        .transpose(1, 0, 2, 3)).astype(bf)
    b_all = np.ascontiguousarray(
        np.concatenate([np.asarray(inputs["b_cls"]), np.asarray(inputs["b_r1"]),
                        np.asarray(inputs["b_r2"])]), f32)
    b_det = np.ascontiguousarray(np.asarray(inputs["b_det"]), f32)
    boxes = np.asarray(inputs["ss_boxes"], f32)[:, 1:5]
    isw = np.ascontiguousarray(np.asarray(inputs["IS_weight"])[:, 0], f32)
    lab = np.ascontiguousarray(np.asarray(inputs["image_level_label"]), np.int32)
    roi = np.asarray(inputs["fc7_roi"], f32).T.astype(bf)
    frm = np.asarray(inputs["fc7_frame"], f32).T.astype(bf)
    ctxm = np.asarray(inputs["fc7_context"], f32).T.astype(bf)

    def _pack(a, sl):
        # [F, NS] slice -> [128, KT, NS]: 8KB-contiguous per-partition runs
        return np.ascontiguousarray(
            a[:, sl].reshape(KT, 128, -1).transpose(1, 0, 2))

    in_maps = []
    for c in range(n_cores):
        sl = slice(c * NS, (c + 1) * NS)
        bsl = boxes[sl]
        isl = isw[sl]
        in_maps.append({
            "roi": _pack(roi, sl),
            "frm": _pack(frm, sl),
            "ctxm": _pack(ctxm, sl),
            "w_all": w_all, "wdc": wdc, "b_all": b_all, "b_det": b_det,
            "boxes": np.ascontiguousarray(
                bsl.reshape(NB, 128, 4).transpose(1, 0, 2)),
            "isw": isl,
            "iswc": np.ascontiguousarray(isl.reshape(NB, 128).T),
            "lab": lab,
        })
    return in_maps


_PROG_CACHE = {}


def _get_prog(NS, F, n_cores):
    key = (NS, F, n_cores)
    if key not in _PROG_CACHE:
        _PROG_CACHE[key] = build_program(NS, F, n_cores)
    return _PROG_CACHE[key]


def kernel(**inputs):
    n_cores = 8
    N, F = inputs["fc7_roi"].shape
    NS = N // n_cores
    prog = _get_prog(NS, F, n_cores)
    in_maps = make_in_maps(inputs, NS, n_cores)
    res = run_bass_kernel_spmd(prog, in_maps, list(range(n_cores))).results
    return np.float32(res[0]["loss"].reshape(()))


# revision 31
# speedup vs baseline: 1.7758x; 1.0649x over previous
# Trainium2 Bass kernel for nn_Network_515396076038 (nms_detection / OICR-style loss).
#
# Strategy (8 NeuronCores, data-parallel over the N=4096 proposals):
#   - Each core owns NS = N/8 = 512 rois and streams its shard of fc7_roi /
#     fc7_frame / fc7_context from HBM in bf16 (12 MB per core -> memory
#     roofline ~34us), computing the GEMM heads in class-major layout [C, NS]
#     (scores^T = W^T @ X^T accumulated over F).  det = W@frm - W@ctx is
#     computed on the PE by accumulating ctx with -W (host-negated weights),
#     so the vector engine does no work during the stream.
#   - Per-class argmax over rois is computed in LOG domain:
#     log p = cls + det + ln(isw) - ln(sum_c exp(cls)) -- the global det
#     softmax normalizer is a per-class constant and cannot change the
#     argmax, and the log form avoids expensive vector reciprocals.
#   - One AllGather ships each core's per-class maxima + candidate boxes
#     (sel-mask matmul gather) + softmax partial sums; refine-head
#     log-softmax prep fills the collective latency.  A final tiny
#     AllReduce(add) sums 16 loss partials (4 roi-blocks x 2 supervisions x
#     {num, den}); the per-block sums ride the collective for free.
#   - IoU / fg-bg assignment / one-hot log-prob gather run roi-major,
#     batched across all 4 roi-blocks and both supervisions in single
#     vector ops on [128, 160] tiles via stride-0 broadcast views.
import sys

for _p in ("/opt/trn_rl_repo",):
    if _p not in sys.path:
        sys.path.append(_p)

import numpy as np
import ml_dtypes

import concourse.bass as bass
import concourse.bass_isa as bass_isa
import concourse.mybir as mybir
import concourse.tile as tile
from concourse import bacc
from concourse.bass_utils import run_bass_kernel_spmd
from concourse.masks import make_identity

dt = mybir.dt
Alu = mybir.AluOpType
Act = mybir.ActivationFunctionType
AX = mybir.AxisListType

C = 20      # foreground classes
CR = C + 1  # refine head classes (background + C)
CA = C + 2 * CR  # stacked roi-head outputs: cls | r1 | r2 = 62
W2 = 2 * C


def _emit(nc, tc, aps, NS, F, n_cores):
    NB = NS // 128
    KT = F // 128
    group = [list(range(n_cores))]
    GW = 241  # AllGather row: vm1[20] vm2[21] boxes[160] z[20] s1[20]

    (roi, frm, ctxm, w_all, wdc, b_all, b_det, boxes, isw, iswc, lab,
     loss) = aps

    const = tc.alloc_tile_pool(name="const", bufs=1)
    st = tc.alloc_tile_pool(name="st", bufs=1)
    stp = tc.alloc_tile_pool(name="stp", bufs=2)
    natp = tc.alloc_tile_pool(name="natp", bufs=2)
    pst = tc.alloc_tile_pool(name="pst", bufs=2, space="PSUM")
    pss1 = tc.alloc_tile_pool(name="pss1", bufs=2, space="PSUM")
    pss2 = tc.alloc_tile_pool(name="pss2", bufs=1, space="PSUM")
    dp = tc.alloc_tile_pool(name="dp", bufs=1, space="DRAM")
    # psc is created last: it is the first pool released (LIFO pool stack)
    psc = tc.alloc_tile_pool(name="psc", bufs=1, space="PSUM")

    # ---- dummy collective: absorbs CC-engine warmup under the DMA stream
    dz = const.tile([1, 1], dt.float32)
    nc.vector.memset(dz, 0.0)
    cc0_in = dp.tile([4], dt.float32)
    cc0_out = dp.tile([4 * n_cores], dt.float32)
    nc.sync.dma_start(cc0_in[0:1], dz[:, 0])
    nc.sync.dma_start(cc0_in[1:2], dz[:, 0])
    nc.sync.dma_start(cc0_in[2:3], dz[:, 0])
    nc.sync.dma_start(cc0_in[3:4], dz[:, 0])
    nc.gpsimd.collective_compute(
        "AllGather", Alu.bypass, replica_groups=group,
        ins=[cc0_in.opt()], outs=[cc0_out.opt()],
    )

    # ---- big stream DMAs first: weights + first super-tile ----
    KS = min(8, KT)
    SK = KT // KS
    w_all_sb = const.tile([128, KT, CA], dt.bfloat16)
    nc.sync.dma_start(w_all_sb, w_all)
    wdc_sb = const.tile([128, KT, 2, C], dt.bfloat16)
    nc.sync.dma_start(wdc_sb, wdc)
    nat0 = []
    for nm, srcp in (("roi", roi), ("frm", frm), ("ctx", ctxm)):
        t = natp.tile([128, KS, NS], dt.bfloat16, tag=nm)
        nc.sync.dma_start(t, srcp[:, bass.ts(0, KS), :])
        nat0.append(t)

    # ---------------- constants / setup (overlaps the GEMM stream) --------
    ident = const.tile([128, 128], dt.float32)
    make_identity(nc, ident)
    ones_col = const.tile([128, 1], dt.float32)
    nc.vector.memset(ones_col, 1.0)
    ones_row = const.tile([1, 128], dt.float32)
    nc.vector.memset(ones_row, 1.0)
    iota_i = const.tile([128, CR], dt.int32)
    nc.gpsimd.iota(iota_i, pattern=[[1, CR]], base=0, channel_multiplier=0)
    iota_f = const.tile([128, CR], dt.float32)
    nc.vector.tensor_copy(iota_f, iota_i)
    iota_m1k = const.tile([128, C], dt.float32)
    nc.vector.tensor_scalar_add(iota_m1k, iota_f[:, :C], -1000.0)
    iota8 = const.tile([128, 2 * NB * CR], dt.float32)
    nc.vector.tensor_copy(
        iota8.rearrange("p (b c) -> p b c", b=2 * NB),
        iota_f[:, None, :].to_broadcast([128, 2 * NB, CR]),
    )
    # pre-warm the exp activation table while the stream runs
    pwz = const.tile([1, 1], dt.float32)
    nc.vector.memset(pwz, 0.0)
    pw = const.tile([1, 1], dt.float32)
    nc.scalar.activation(pw, pwz, Act.Exp)

    b_all_sb = const.tile([CA, 1], dt.float32)
    nc.sync.dma_start(b_all_sb, b_all[:, None])
    b_det_sb = const.tile([C, 1], dt.float32)
    nc.sync.dma_start(b_det_sb, b_det[:, None])

    labrow_i = st.tile([1, C], dt.int32)
    nc.sync.dma_start(labrow_i, lab)
    labrow_f = st.tile([1, C], dt.float32)
    nc.vector.tensor_copy(labrow_f, labrow_i)
    mask_row = st.tile([1, W2], dt.float32)
    nc.vector.tensor_scalar(mask_row[:, 0:C], labrow_f, 1.0, None, Alu.is_equal)
    nc.vector.tensor_copy(mask_row[:, C:W2], mask_row[:, 0:C])

    isw_row = st.tile([1, NS], dt.float32)
    nc.sync.dma_start(isw_row, isw[None, :])
    isw_col = st.tile([128, NB], dt.float32)
    nc.sync.dma_start(isw_col, iswc)
    boxes_nat = st.tile([128, NB, 4], dt.float32)
    nc.sync.dma_start(boxes_nat, boxes)

    # per-roi box areas [128, NB]
    ab_all = st.tile([128, NB], dt.float32)
    abt = stp.tile([128, NB], dt.float32, tag="abt")
    nc.vector.scalar_tensor_tensor(
        abt, boxes_nat[:, :, 2], 1.0, boxes_nat[:, :, 0], Alu.add, Alu.subtract)
    nc.vector.scalar_tensor_tensor(
        ab_all, boxes_nat[:, :, 3], 1.0, boxes_nat[:, :, 1], Alu.add, Alu.subtract)
    nc.vector.tensor_mul(ab_all, ab_all, abt)

    # ---------------- main GEMM phase (bf16, DMA-bound) ----------------
    # scoresA rows: [0:C] cls, [C:C+CR] r1, [C+CR:CA] r2 ; scoresB rows [0:C] det
    scoresA = psc.tile([128, NS], dt.float32)
    scoresB = psc.tile([128, NS], dt.float32)

    for sk in range(SK):
        if sk == 0:
            t_roi, t_frm, t_ctx = nat0
        else:
            ksl = bass.ts(sk, KS)
            t_roi = natp.tile([128, KS, NS], dt.bfloat16, tag="roi")
            nc.sync.dma_start(t_roi, roi[:, ksl, :])
            t_frm = natp.tile([128, KS, NS], dt.bfloat16, tag="frm")
            nc.sync.dma_start(t_frm, frm[:, ksl, :])
            t_ctx = natp.tile([128, KS, NS], dt.bfloat16, tag="ctx")
            nc.sync.dma_start(t_ctx, ctxm[:, ksl, :])
        for j in range(KS):
            k = sk * KS + j
            nc.tensor.matmul(
                scoresA[0:CA, :], w_all_sb[:, k, :], t_roi[:, j, :],
                start=(k == 0), stop=(k == KT - 1),
            )
        for j in range(KS):
            k = sk * KS + j
            nc.tensor.matmul(
                scoresB[0:C, :], wdc_sb[:, k, 0, :], t_frm[:, j, :],
                start=(k == 0), stop=False,
            )
        for j in range(KS):
            k = sk * KS + j
            nc.tensor.matmul(
                scoresB[0:C, :], wdc_sb[:, k, 1, :], t_ctx[:, j, :],
                start=False, stop=(k == KT - 1),
            )

    # ---------------- class-major stats (log-domain argmax) ----------------
    stk = st.tile([CA, NS], dt.float32)
    nc.scalar.activation(stk, scoresA[0:CA, :], Act.Identity, bias=b_all_sb)
    det_sb = st.tile([C, NS], dt.float32)
    nc.vector.tensor_scalar(det_sb, scoresB[0:C, :], b_det_sb, None, Alu.add)
    psc.release()
    # de-stack r1/r2 to partition base 0 (SBUF->SBUF DMA moves partitions)
    r1_sb = st.tile([CR, NS], dt.float32)
    nc.sync.dma_start(r1_sb, stk[C:C + CR, :])
    r2_sb = st.tile([CR, NS], dt.float32)
    nc.sync.dma_start(r2_sb, stk[C + CR:CA, :])

    # vmzs cols: 0 = per-class max (filled later), 1 = z partial, 2 = s1
    vmzs = st.tile([C, 3], dt.float32)
    exp_det = st.tile([C, NS], dt.float32)
    nc.scalar.activation(exp_det, det_sb, Act.Exp, accum_out=vmzs[:, 1:2])
    prod_cd = st.tile([C, NS], dt.float32)
    nc.vector.tensor_mul(prod_cd, stk[0:C, :], exp_det)
    nc.vector.reduce_sum(vmzs[:, 2:3], prod_cd, axis=AX.X)

    exp_cls = st.tile([C, NS], dt.float32)
    nc.scalar.activation(exp_cls, stk[0:C, :], Act.Exp)
    exp_r1 = st.tile([CR, NS], dt.float32)
    nc.scalar.activation(exp_r1, r1_sb, Act.Exp)

    ps_s1 = pss1.tile([128, NS], dt.float32, tag="mm")
    nc.tensor.matmul(ps_s1[0:1, 0:NS], ones_col[0:C, :], exp_cls,
                     start=True, stop=True)
    ps_s2 = pss1.tile([128, NS], dt.float32, tag="mm")
    nc.tensor.matmul(ps_s2[0:1, 0:NS], ones_col[0:CR, :], exp_r1,
                     start=True, stop=True)
    # trow = ln(isw) - ln(normalizer), per head.  Both normalizers go
    # through ONE Ln activation so the scheduler cannot interleave exp/ln
    # table loads (the combined Ln depends on both exp matmuls).
    ln_isw = st.tile([1, NS], dt.float32)
    nc.scalar.activation(ln_isw, isw_row, Act.Ln)
    lnin = st.tile([1, 2 * NS], dt.float32)
    nc.vector.tensor_copy(lnin[:, 0:NS], ps_s1[0:1, 0:NS])
    nc.vector.tensor_copy(lnin[:, NS:2 * NS], ps_s2[0:1, 0:NS])
    lnout = st.tile([1, 2 * NS], dt.float32)
    nc.scalar.activation(lnout, lnin, Act.Ln)
    trow1 = st.tile([1, NS], dt.float32)
    nc.vector.tensor_sub(trow1, ln_isw, lnout[:, 0:NS])
    trow2 = st.tile([1, NS], dt.float32)
    nc.vector.tensor_sub(trow2, ln_isw, lnout[:, NS:2 * NS])

    ps_b1 = pss1.tile([128, NS], dt.float32, tag="mm")
    nc.tensor.matmul(ps_b1[0:C, 0:NS], ones_row[:, 0:C], trow1,
                     start=True, stop=True)
    ps_b2 = pss1.tile([128, NS], dt.float32, tag="mm")
    nc.tensor.matmul(ps_b2[0:CR, 0:NS], ones_row[:, 0:CR], trow2,
                     start=True, stop=True)

    # lp1 = cls + det + trow1 ; lq2 = r1 + trow2  (argmax-equivalent logs)
    lp1 = st.tile([C, NS], dt.float32)
    nc.vector.tensor_add(lp1, stk[0:C, :], det_sb)
    nc.vector.tensor_add(lp1, lp1, ps_b1[0:C, 0:NS])
    lq2 = st.tile([CR, NS], dt.float32)
    nc.vector.tensor_add(lq2, r1_sb, ps_b2[0:CR, 0:NS])

    nc.vector.reduce_max(vmzs[:, 0:1], lp1, axis=AX.X)
    vm1 = vmzs[:, 0:1]
    vm2 = st.tile([CR, 1], dt.float32)
    nc.vector.reduce_max(vm2, lq2, axis=AX.X)

    # local per-class argmax boxes via sel-mask matmuls (exact one-hot gather)
    sel1 = st.tile([C, NS], dt.float32)
    nc.vector.tensor_scalar(sel1, lp1, vmzs[:, 0:1], None, Alu.is_equal)
    sel2 = st.tile([CR, NS], dt.float32)
    nc.vector.tensor_scalar(sel2, lq2, vm2, None, Alu.is_equal)
    psq = pss2.tile([128, 64], dt.float32, tag="acc")
    for b in range(NB):
        bsl = bass.ts(b, 128)
        ptx = pst.tile([128, 64], dt.float32, tag="pt")
        nc.tensor.transpose(ptx[:, 0:C], sel1[:, bsl], ident[0:C, 0:C])
        nc.tensor.transpose(ptx[:, 32:32 + CR], sel2[:, bsl], ident[0:CR, 0:CR])
        sT = stp.tile([128, 64], dt.float32, tag="sT")
        nc.vector.tensor_copy(sT[:, 0:C], ptx[:, 0:C])
        nc.vector.tensor_copy(sT[:, 32:32 + CR], ptx[:, 32:32 + CR])
        nc.tensor.matmul(
            psq[0:4, 0:C], boxes_nat[:, b, :], sT[:, 0:C],
            start=(b == 0), stop=(b == NB - 1), skip_group_check=True,
        )
        nc.tensor.matmul(
            psq[0:4, C:W2], boxes_nat[:, b, :], sT[:, 33:33 + C],
            start=(b == 0), stop=(b == NB - 1), skip_group_check=True,
        )
    bc_sb = st.tile([4, W2], dt.float32)
    nc.vector.tensor_copy(bc_sb, psq[0:4, 0:W2])

    # ---------------- G1: AllGather of all cross-core state ----------------
    g1_in = dp.tile([GW], dt.float32)
    g1_out = dp.tile([n_cores * GW], dt.float32)
    nc.sync.dma_start(g1_in[0:60], vmzs)
    nc.sync.dma_start(g1_in[60:81], vm2[:, 0])
    nc.sync.dma_start(g1_in[81:241], bc_sb)
    nc.gpsimd.collective_compute(
        "AllGather", Alu.bypass, replica_groups=group,
        ins=[g1_in.opt()], outs=[g1_out.opt()],
    )

    # ---- collective-independent prep, emitted here to fill G1 latency ----
    # refine-head scores to roi-major [128, (b,s), CR], then log-softmax
    rts = st.tile([128, NB * 2 * CR], dt.float32)
    for b in range(NB):
        bsl = bass.ts(b, 128)
        ptr = pst.tile([128, 64], dt.float32, tag="pt")
        nc.tensor.transpose(ptr[:, 0:CR], r1_sb[:, bsl], ident[0:CR, 0:CR])
        nc.tensor.transpose(ptr[:, CR:2 * CR], r2_sb[:, bsl], ident[0:CR, 0:CR])
        nc.vector.tensor_copy(rts[:, b * 2 * CR:(b + 1) * 2 * CR], ptr[:, 0:2 * CR])
    rts3 = rts.rearrange("p (g c) -> p g c", g=2 * NB)
    rmax = st.tile([128, 2 * NB], dt.float32)
    nc.vector.reduce_max(rmax, rts3, axis=AX.X)
    xs_all = st.tile([128, NB * 2 * CR], dt.float32)
    xs3 = xs_all.rearrange("p (g c) -> p g c", g=2 * NB)
    nc.vector.tensor_tensor(
        xs3, rts3, rmax[:, :, None].to_broadcast([128, 2 * NB, CR]), Alu.subtract)
    ex_all = st.tile([128, NB * 2 * CR], dt.float32)
    nc.scalar.activation(ex_all, xs_all, Act.Exp)
    ssum = st.tile([128, 2 * NB], dt.float32)
    nc.vector.reduce_sum(ssum, ex_all.rearrange("p (g c) -> p g c", g=2 * NB),
                         axis=AX.X)
    lse = st.tile([128, 2 * NB], dt.float32)
    nc.scalar.activation(lse, ssum, Act.Ln)
    nc.vector.tensor_tensor(
        xs3, xs3, lse[:, :, None].to_broadcast([128, 2 * NB, CR]), Alu.subtract)
    # xs_all now holds log-probs for both refine heads

    # ---------------- G1 readback + cross-core combine ----------------
    g_sb = st.tile([n_cores, GW], dt.float32)
    nc.sync.dma_start(g_sb, g1_out.rearrange("(r w) -> r w", r=n_cores))
    vmx = st.tile([n_cores, 81], dt.float32)
    nc.gpsimd.partition_all_reduce(
        vmx, g_sb[:, 0:81], channels=n_cores, reduce_op=bass_isa.ReduceOp.max
    )
    selc = st.tile([n_cores, 81], dt.float32)
    nc.vector.tensor_tensor(selc, g_sb[:, 0:81], vmx, Alu.is_equal)
    sel1m = st.tile([n_cores, C], dt.float32)
    nc.vector.tensor_copy(
        sel1m[:, :, None],
        selc[:, 0:60].rearrange("r (c t) -> r c t", t=3)[:, :, 0:1])
    masked = st.tile([n_cores, 160], dt.float32)
    mview = masked.rearrange("p (co s c) -> p co s c", co=4, s=2)
    gview = g_sb[:, 81:241].rearrange("p (co s c) -> p co s c", co=4, s=2)
    nc.vector.tensor_tensor(
        mview[:, :, 0, :], gview[:, :, 0, :],
        sel1m[:, None, :].to_broadcast([n_cores, 4, C]), Alu.mult,
    )
    nc.vector.tensor_tensor(
        mview[:, :, 1, :], gview[:, :, 1, :],
        selc[:, None, 61:81].to_broadcast([n_cores, 4, C]), Alu.mult,
    )
    ps_qr = pss1.tile([128, NS], dt.float32, tag="mm")
    nc.tensor.matmul(ps_qr[0:1, 0:160], ones_col[0:n_cores, :], masked,
                     start=True, stop=True, skip_group_check=True)
    nc.tensor.matmul(ps_qr[0:1, 160:220], ones_col[0:n_cores, :], g_sb[:, 0:60],
                     start=True, stop=True, skip_group_check=True)
    qzs = st.tile([1, 220], dt.float32)
    nc.vector.tensor_copy(qzs, ps_qr[0:1, 0:220])

    # broadcast [boxes(160) | mask(40)] to all 128 partitions
    ps_q = pss1.tile([128, NS], dt.float32, tag="mm")
    nc.tensor.matmul(ps_q[:, 0:160], ones_row[0:1, :], qzs[:, 0:160],
                     start=True, stop=True, skip_group_check=True)
    nc.tensor.matmul(ps_q[:, 160:200], ones_row[0:1, :], mask_row,
                     start=True, stop=True, skip_group_check=True)
    # materialize NB-tiled copy so later ops use plain strided in0 views
    Q4 = st.tile([128, NB, 200], dt.float32)
    nc.vector.tensor_copy(
        Q4, ps_q[:, None, 0:200].to_broadcast([128, NB, 200]))

    # query areas + roi areas [128, NB, 40]
    aqt = stp.tile([128, NB, W2], dt.float32, tag="aqt")
    nc.vector.scalar_tensor_tensor(
        aqt, Q4[:, :, 80:120], 1.0, Q4[:, :, 0:40], Alu.add, Alu.subtract)
    ab40 = st.tile([128, NB, W2], dt.float32)
    nc.vector.scalar_tensor_tensor(
        ab40, Q4[:, :, 120:160], 1.0, Q4[:, :, 40:80], Alu.add, Alu.subtract)
    nc.vector.tensor_mul(ab40, ab40, aqt)
    nc.vector.tensor_tensor(
        ab40, ab40, ab_all[:, :, None].to_broadcast([128, NB, W2]), Alu.add)

    # ---------------- batched paired IoU / assignment / loss ----------------
    xi1 = stp.tile([128, NB, W2], dt.float32, tag="xi1")
    nc.vector.tensor_tensor(
        xi1, Q4[:, :, 0:40],
        boxes_nat[:, :, 0:1].to_broadcast([128, NB, W2]), Alu.max)
    yi1 = stp.tile([128, NB, W2], dt.float32, tag="yi1")
    nc.vector.tensor_tensor(
        yi1, Q4[:, :, 40:80],
        boxes_nat[:, :, 1:2].to_broadcast([128, NB, W2]), Alu.max)
    xi2 = stp.tile([128, NB, W2], dt.float32, tag="xi2")
    nc.vector.tensor_tensor(
        xi2, Q4[:, :, 80:120],
        boxes_nat[:, :, 2:3].to_broadcast([128, NB, W2]), Alu.min)
    yi2 = stp.tile([128, NB, W2], dt.float32, tag="yi2")
    nc.vector.tensor_tensor(
        yi2, Q4[:, :, 120:160],
        boxes_nat[:, :, 3:4].to_broadcast([128, NB, W2]), Alu.min)
    nc.vector.scalar_tensor_tensor(xi2, xi2, 1.0, xi1, Alu.add, Alu.subtract)
    nc.vector.tensor_scalar_max(xi2, xi2, 0.0)   # iw
    nc.vector.scalar_tensor_tensor(yi2, yi2, 1.0, yi1, Alu.add, Alu.subtract)
    nc.vector.tensor_scalar_max(yi2, yi2, 0.0)   # ih
    inter = stp.tile([128, NB, W2], dt.float32, tag="inter")
    nc.vector.tensor_mul(inter, xi2, yi2)
    un = stp.tile([128, NB, W2], dt.float32, tag="un")
    nc.vector.tensor_sub(un, ab40, inter)
    unf = un.rearrange("p b w -> p (b w)")
    nc.vector.reciprocal(unf, unf)
    ov = stp.tile([128, NB, W2], dt.float32, tag="ov")
    nc.vector.tensor_mul(ov, inter, un)
    # mask image-level negatives to exactly -1: ov = (ov+1)*mask - 1
    nc.vector.scalar_tensor_tensor(
        ov, ov, 1.0, Q4[:, :, 160:200], Alu.add, Alu.mult)
    ovf = ov.rearrange("p b w -> p (b w)")
    nc.vector.tensor_scalar_add(ovf, ovf, -1.0)

    ov4 = ov.rearrange("p b (s c) -> p (b s) c", s=2)
    mo = stp.tile([128, 2 * NB], dt.float32, tag="mo")
    nc.vector.reduce_max(mo, ov4, axis=AX.X)
    meq = stp.tile([128, 2 * NB, C], dt.float32, tag="meq")
    nc.vector.tensor_tensor(
        meq, ov4, mo[:, :, None].to_broadcast([128, 2 * NB, C]), Alu.is_equal)
    nc.vector.tensor_tensor(
        meq, meq, iota_m1k[:, None, :].to_broadcast([128, 2 * NB, C]), Alu.mult)
    gt = stp.tile([128, 2 * NB], dt.float32, tag="gt")
    nc.vector.tensor_reduce(gt, meq, axis=AX.X, op=Alu.min)

    fg = stp.tile([128, 2 * NB], dt.float32, tag="fg")
    nc.vector.tensor_scalar(fg, mo, 0.5, None, Alu.is_gt)
    bgt = stp.tile([128, 2 * NB], dt.float32, tag="bgt")
    nc.vector.tensor_scalar(bgt, mo, 0.5, None, Alu.is_lt)
    bg = stp.tile([128, 2 * NB], dt.float32, tag="bg")
    nc.vector.scalar_tensor_tensor(bg, mo, 0.1, bgt, Alu.is_ge, Alu.mult)
    keep = stp.tile([128, 2 * NB], dt.float32, tag="keep")
    nc.vector.tensor_add(keep, fg, bg)
    col = stp.tile([128, 2 * NB], dt.float32, tag="col")
    nc.vector.scalar_tensor_tensor(col, gt, 1001.0, fg, Alu.add, Alu.mult)

    oh = stp.tile([128, 2 * NB, CR], dt.float32, tag="oh")
    nc.vector.tensor_tensor(
        oh, iota8.rearrange("p (g c) -> p g c", g=2 * NB),
        col[:, :, None].to_broadcast([128, 2 * NB, CR]), Alu.is_equal)
    nc.vector.tensor_tensor(oh, oh, xs3, Alu.mult)
    lpsel = stp.tile([128, 2 * NB], dt.float32, tag="lpsel")
    nc.vector.reduce_sum(lpsel, oh, axis=AX.X)

    wk = st.tile([128, 16], dt.float32)
    wl3 = wk[:, 0:8].rearrange("p (b s) -> p b s", s=2)
    nc.vector.tensor_tensor(
        wl3, keep.rearrange("p (b s) -> p b s", s=2),
        isw_col[:, :, None].to_broadcast([128, NB, 2]), Alu.mult)
    nc.vector.tensor_tensor(wk[:, 0:8], wk[:, 0:8], lpsel, Alu.mult)
    nc.vector.tensor_copy(wk[:, 8:16], keep)
    ps_l = pss2.tile([128, 64], dt.float32, tag="acc")
    nc.tensor.matmul(ps_l[0:16, 0:1], wk, ones_col, start=True, stop=True)
    l16 = st.tile([16, 1], dt.float32)
    nc.vector.tensor_copy(l16, ps_l[0:16, 0:1])

    # ---------------- R3: AllGather of loss partials ----------------
    # (an 8-core AllGather is a 2-event mesh vs AllReduce's 5 events; the
    # cross-core sum happens locally with one ones-matmul)
    cc3_in = dp.tile([16], dt.float32)
    cc3_out = dp.tile([n_cores * 16], dt.float32)
    nc.sync.dma_start(cc3_in, l16[:, 0])
    nc.gpsimd.collective_compute(
        "AllGather", Alu.bypass, replica_groups=group,
        ins=[cc3_in.opt()], outs=[cc3_out.opt()],
    )

    # ---- hinge loss from qzs (fills R3 latency) ----
    zsv = qzs[:, 160:220].rearrange("a (c t) -> a c t", t=3)
    zrow = st.tile([1, C], dt.float32)
    nc.vector.tensor_copy(zrow[:, :, None], zsv[:, :, 1:2])
    s1row = st.tile([1, C], dt.float32)
    nc.vector.tensor_copy(s1row[:, :, None], zsv[:, :, 2:3])
    zinv = st.tile([1, C], dt.float32)
    nc.vector.reciprocal(zinv, zrow)
    dcs = st.tile([1, C], dt.float32)
    nc.vector.tensor_mul(dcs, s1row, zinv)
    hv = st.tile([1, C], dt.float32)
    nc.vector.tensor_mul(hv, labrow_f, dcs)
    nc.scalar.activation(hv, hv, Act.Relu, bias=1.0, scale=-1.0)  # relu(1-lab*dcs)
    h = st.tile([1, 1], dt.float32)
    nc.vector.reduce_sum(h, hv, axis=AX.X)
    nc.scalar.mul(h, h, 1.0 / C)

    # ---------------- R3 readback + final scalar ----------------
    g2_sb = st.tile([n_cores, 16], dt.float32)
    nc.sync.dma_start(g2_sb, cc3_out.rearrange("(r w) -> r w", r=n_cores))
    ps_f = pss1.tile([128, NS], dt.float32, tag="mm")
    nc.tensor.matmul(ps_f[0:1, 0:16], ones_col[0:n_cores, :], g2_sb,
                     start=True, stop=True)
    l4 = st.tile([1, 16], dt.float32)
    nc.vector.tensor_copy(l4, ps_f[0:1, 0:16])
    nums = st.tile([1, 2], dt.float32)
    nc.vector.reduce_sum(
        nums, l4[:, 0:8].rearrange("a (b s) -> a s b", s=2), axis=AX.X)
    dens = st.tile([1, 2], dt.float32)
    nc.vector.reduce_sum(
        dens, l4[:, 8:16].rearrange("a (b s) -> a s b", s=2), axis=AX.X)
    dinv = st.tile([1, 2], dt.float32)
    nc.vector.reciprocal(dinv, dens)
    rl = st.tile([1, 2], dt.float32)
    nc.vector.tensor_mul(rl, nums, dinv)
    rsum = st.tile([1, 1], dt.float32)
    nc.vector.reduce_sum(rsum, rl, axis=AX.X)
    tot = st.tile([1, 1], dt.float32)
    nc.scalar.mul(tot, rsum, -0.1)
    nc.vector.tensor_add(tot, tot, h)
    nc.sync.dma_start(loss, tot)

    for pool in (dp, pss2, pss1, pst, natp, stp, st, const):
        pool.release()


def build_program(NS=512, F=4096, n_cores=8):
    nc = bacc.Bacc(
        "TRN2", target_bir_lowering=False, debug=False, num_devices=n_cores
    )
    KT = F // 128
    NB = NS // 128
    bf = dt.bfloat16
    f32 = dt.float32
    roi = nc.dram_tensor("roi", [128, KT, NS], bf, kind="ExternalInput").ap()
    frm = nc.dram_tensor("frm", [128, KT, NS], bf, kind="ExternalInput").ap()
    ctxm = nc.dram_tensor("ctxm", [128, KT, NS], bf, kind="ExternalInput").ap()
    w_all = nc.dram_tensor("w_all", [128, KT, CA], bf, kind="ExternalInput").ap()
    wdc = nc.dram_tensor("wdc", [128, KT, 2, C], bf, kind="ExternalInput").ap()
    b_all = nc.dram_tensor("b_all", [CA], f32, kind="ExternalInput").ap()
    b_det = nc.dram_tensor("b_det", [C], f32, kind="ExternalInput").ap()
    boxes = nc.dram_tensor("boxes", [128, NB, 4], f32, kind="ExternalInput").ap()
    isw = nc.dram_tensor("isw", [NS], f32, kind="ExternalInput").ap()
    iswc = nc.dram_tensor("iswc", [128, NB], f32, kind="ExternalInput").ap()
    lab = nc.dram_tensor("lab", [1, C], dt.int32, kind="ExternalInput").ap()
    loss = nc.dram_tensor("loss", [1, 1], f32, kind="ExternalOutput").ap()
    aps = (roi, frm, ctxm, w_all, wdc, b_all, b_det, boxes, isw, iswc, lab,
           loss)
    with tile.TileContext(nc) as tc:
        _emit(nc, tc, aps, NS, F, n_cores)
    nc.compile()
    return nc


def make_in_maps(inputs, NS, n_cores):
    f32 = np.float32
    bf = ml_dtypes.bfloat16
    F = inputs["fc7_roi"].shape[1]
    KT = F // 128
    NB = NS // 128

    def _wpack(w):
        # [F, cols] -> [128, KT, cols] contiguous
        return np.ascontiguousarray(
            np.asarray(w, f32).reshape(KT, 128, -1).transpose(1, 0, 2)
        ).astype(bf)

    w_all = _wpack(np.concatenate(
        [np.asarray(inputs["W_cls"]), np.asarray(inputs["W_r1"]),
         np.asarray(inputs["W_r2"])], axis=1))
    wd = np.asarray(inputs["W_det"], f32)
    wdc = np.ascontiguousarray(
        np.stack([wd.reshape(KT, 128, C), -wd.reshape(KT, 128, C)], axis=2)
        .transpose(1, 0, 2, 3)).astype(bf)
    b_all = np.ascontiguousarray(
        np.concatenate([np.asarray(inputs["b_cls"]), np.asarray(inputs["b_r1"]),
                        np.asarray(inputs["b_r2"])]), f32)
    b_det = np.ascontiguousarray(np.asarray(inputs["b_det"]), f32)
    boxes = np.asarray(inputs["ss_boxes"], f32)[:, 1:5]
    isw = np.ascontiguousarray(np.asarray(inputs["IS_weight"])[:, 0], f32)
    lab = np.ascontiguousarray(np.asarray(inputs["image_level_label"]), np.int32)
    roi = np.asarray(inputs["fc7_roi"], f32).T.astype(bf)
    frm = np.asarray(inputs["fc7_frame"], f32).T.astype(bf)
    ctxm = np.asarray(inputs["fc7_context"], f32).T.astype(bf)

    def _pack(a, sl):
        # [F, NS] slice -> [128, KT, NS]: 8KB-contiguous per-partition runs
        return np.ascontiguousarray(
            a[:, sl].reshape(KT, 128, -1).transpose(1, 0, 2))

    in_maps = []
    for c in range(n_cores):
        sl = slice(c * NS, (c + 1) * NS)
        bsl = boxes[sl]
        isl = isw[sl]
        in_maps.append({
            "roi": _pack(roi, sl),
            "frm": _pack(frm, sl),
            "ctxm": _pack(ctxm, sl),
            "w_all": w_all, "wdc": wdc, "b_all": b_all, "b_det": b_det,
            "boxes": np.ascontiguousarray(
                bsl.reshape(NB, 128, 4).transpose(1, 0, 2)),
            "isw": isl,
            "iswc": np.ascontiguousarray(isl.reshape(NB, 128).T),
            "lab": lab,
        })
    return in_maps


_PROG_CACHE = {}


def _get_prog(NS, F, n_cores):
    key = (NS, F, n_cores)
    if key not in _PROG_CACHE:
        _PROG_CACHE[key] = build_program(NS, F, n_cores)
    return _PROG_CACHE[key]


def kernel(**inputs):
    n_cores = 8
    N, F = inputs["fc7_roi"].shape
    NS = N // n_cores
    prog = _get_prog(NS, F, n_cores)
    in_maps = make_in_maps(inputs, NS, n_cores)
    res = run_bass_kernel_spmd(prog, in_maps, list(range(n_cores))).results
    return np.float32(res[0]["loss"].reshape(()))
